# revision 43
# baseline (speedup 1.0000x reference)
"""Bidirectional Mamba block on 8 Trainium2 NeuronCores.

Sharding: 8 cores = 4 batches x 2 directions (fwd/bwd). Each core runs the
full per-(batch, direction) Mamba pipeline on a time-transposed slice
x[b].T (time-flipped for the backward direction), producing its partial
contribution to the fused output projection. Host sums fwd+bwd partials,
adds the residual and fusion bias.

v2 layout: [d (partitions), t (free)], selective scan restructured:
  - n-loop runs in 2 passes over d-PAIRS using mega tiles [128, 4098]
    (two 2048-column d-tile segments + poisoned boundary columns where
    delta=1e9 -> dA=exp(A*1e9)=0 and u2=0 -> dBu=0, so one
    tensor_tensor_scan instruction covers both segments with a clean
    state reset).
  - B/C broadcast tiles are [128, 2049]; the mega elementwise mults read
    them twice via a stride-0 middle AP dim (keeps DVE 2x mode).
  - y = sum_n h_n*C_n accumulated on the TENSOR engine: per n, 8
    identity matmuls [128x128x512] accumulate yp slices into 8 PSUM
    banks (2 d-tiles x 4 t-chunks). DVE no longer does the adds.
  - gate fused with the PSUM drain (diag(D) close matmul + one TT).

v3 scheduling (the scan window is at the DVE floor of ~13.3us/iter =
dBu TT 2.3 + scan 8.7 + yp TT 2.3; HW rates: TT 2x=0.56ns/el,
tensor_scalar 4x=0.30ns/el, scan 2.12ns/col, Act 0.9ns/el):
  - fus_w @ out_w folded on the host -> single-GEMM epilogue.
  - weight DMAs split across the sync/Act/gpsimd descriptor queues so
    u-proj does not queue behind the x loads.
  - n=0 pipeline fused per chunk into the x_dbl loop with CHAINED
    chunk scans (initial=h[:, prev_last]) - the first scan piece
    issues ~15us before the full-width delta would be ready.
  - z-projection emitted between held n1 and the pair-1 delta; PSUM
    drains via Act so the prologue PSUM frees before psy needs banks.
  - pass-0 drain emitted after pass-1's first compute.
  - PSUM->SBUF drains on Act (scalar.copy); output DMAs on 2 queues.
CAUTION: engine times vary ~20% across device allocations (clock
bins); compare runs via the mega-scan duration (8.69us fast bin).
"""

import numpy as np
import ml_dtypes

import concourse.bass as bass
import concourse.bacc as bacc
import concourse.tile as tile
from concourse import mybir
from concourse.bass_utils import run_bass_kernel_spmd

T = 2048
TP = T + 1          # broadcast tile width (padded)
MW = 2 * T + 2      # mega width: [0:T) seg A, T poison, [T+1:2T+1) seg B, 2T+1 poison
DM = 256      # d_model
DI = 512      # d_inner
DS = 16       # d_state
DR = 16       # dt_rank
NCHUNK = 4    # matmul moving-dim chunks of 512
CH = T // NCHUNK
NDT = DI // 128  # 4 d-tiles of 128 partitions

BF = mybir.dt.bfloat16
F32 = mybir.dt.float32
AF = mybir.ActivationFunctionType
OP = mybir.AluOpType

_CACHE = {}


def _bcast_ap(dram_handle, row, col0, width):
    """AP reading dram[row, col0:col0+width] broadcast across 128 partitions."""
    base = dram_handle[row:row + 1, col0:col0 + width]
    return bass.AP(tensor=base.tensor, offset=base.offset,
                   ap=[[0, 128], [1, width]])


def _rep2_ap(tile_, width, col0=0):
    """Free-replicated read of tile_[:, col0:col0+width] twice."""
    return bass.AP(tensor=tile_.tensor, offset=tile_.offset + col0,
                   ap=[tile_.ap[0], [0, 2], [1, width]])


def _bc2_ap(dram_handle, row, width):
    """Broadcast rows `row` and `row+DS` of bcb as one [128, 2*width] read."""
    base = dram_handle[row:row + 1, 0:width]
    return bass.AP(tensor=base.tensor, offset=base.offset,
                   ap=[[0, 128], [DS * width, 2], [1, width]])


def _build(avals):
    nc = bacc.Bacc()

    # --- I/O ---------------------------------------------------------------
    xt = nc.declare_dram_parameter("xt", [DM, T], BF, isOutput=False)
    inwT = nc.declare_dram_parameter("inwT", [DM, 2 * DI], BF, isOutput=False)
    xpwT = nc.declare_dram_parameter("xpwT", [DI, DR + 2 * DS], BF, isOutput=False)
    dtwT = nc.declare_dram_parameter("dtwT", [DR, DI], BF, isOutput=False)
    # weffT = (fus_w_half @ out_w).T — the fusion matmul folded into the
    # output projection on the host, so the epilogue is a single GEMM
    weffT = nc.declare_dram_parameter("weffT", [DI, DM], BF, isOutput=False)
    convw = nc.declare_dram_parameter("convw", [DI, 4], F32, isOutput=False)
    convb = nc.declare_dram_parameter("convb", [DI, 1], F32, isOutput=False)
    dtb = nc.declare_dram_parameter("dtb", [DI, 1], F32, isOutput=False)
    dvec = nc.declare_dram_parameter("dvec", [DI, 1], F32, isOutput=False)
    idh = nc.declare_dram_parameter("idh", [128, 128], BF, isOutput=False)
    ddg = nc.declare_dram_parameter("ddg", [NDT * 128, 128], BF, isOutput=False)
    o2 = nc.declare_dram_parameter("o2", [DM, T], BF, isOutput=True)

    # DRAM scratch for partition-broadcast bounces
    bcb = nc.dram_tensor("bcb", [2 * DS, TP], BF)   # B rows 0..15, C rows 16..31

    with tile.TileContext(nc) as tc:
        with (
            tc.tile_pool(name="const", bufs=1) as const,
            tc.tile_pool(name="big", bufs=2) as big,
            tc.tile_pool(name="pers", bufs=4) as pers,
            tc.tile_pool(name="work", bufs=2) as work,
            tc.tile_pool(name="nb_", bufs=2) as nbp,
        ):
            # --- load x (chunked across DMA queues) -------------------------
            xn = [big.tile([128, T], BF, tag="xn", name="xn", bufs=2)
                  for _ in range(2)]
            for k in range(2):
                for c in range(NCHUNK):
                    cs = slice(c * CH, (c + 1) * CH)
                    nc.sync.dma_start(out=xn[k][:, cs],
                                      in_=xt[k * 128:(k + 1) * 128, cs])


            ident = const.tile([128, 128], BF, tag="ident", name="ident")
            nc.gpsimd.dma_start(out=ident, in_=idh[:, :])
            w_ddg = [const.tile([128, 128], BF, tag="wddg", name="wddg",
                                bufs=NDT) for _ in range(NDT)]
            for kq in range(NDT):
                nc.sync.dma_start(out=w_ddg[kq], in_=ddg[kq * 128:(kq + 1) * 128, :])
            # zero the padded column of the B/C bounce buffer so the
            # broadcast reads a finite value at the mega poison column
            zrow = const.tile([2 * DS, 1], BF, tag="zrow", name="zrow")
            nc.vector.memset(zrow, 0.0)
            nc.sync.dma_start(out=bcb[:, T:TP], in_=zrow)
            # persistent mega tiles (2 d-pairs)
            u2m = [pers.tile([128, MW], BF, tag="u2m", name="u2m", bufs=2)
                   for _ in range(2)]
            dlm = [pers.tile([128, MW], BF, tag="dlm", name="dlm", bufs=2)
                   for _ in range(2)]
            wdm = [pers.tile([128, MW], BF, tag="wdm", name="wdm", bufs=2)
                   for _ in range(2)]
            sz = [pers.tile([128, T], BF, tag="sz", name="sz") for _ in range(NDT)]
            ygc = [pers.tile([128, T], BF, tag="ygc", name="ygc")
                   for _ in range(NDT)]
            # poison columns: delta=1e9, u2=0, wdm=0 at cols T and 2T+1
            # (wdm poisons are memset because the chunked pair-0 fill never
            # touches them, and stale NaN*0 = NaN would break the scan reset)
            for p in range(2):
                nc.vector.memset(dlm[p][:, T:T + 1], 1e9)
                nc.vector.memset(dlm[p][:, MW - 1:MW], 1e9)
                nc.vector.memset(u2m[p][:, T:T + 1], 0.0)
                nc.vector.memset(u2m[p][:, MW - 1:MW], 0.0)
                nc.vector.memset(wdm[p][:, T:T + 1], 0.0)
                nc.vector.memset(wdm[p][:, MW - 1:MW], 0.0)

            def mseg(p, k):
                """Segment slice of mega tile for d-tile index (2*p + k)."""
                return slice(k * (T + 1), k * (T + 1) + T)

            with tc.tile_pool(name="ps", bufs=2, space="PSUM") as ps, \
                 tc.tile_pool(name="pss", bufs=2, space="PSUM") as pss:
                # --- weights/constants ------------------------------------------
                w_inwT = [const.tile([128, 2 * DI], BF, tag="winw", name="winw",
                                     bufs=2) for _ in range(2)]
                for k in range(2):
                    nc.gpsimd.dma_start(out=w_inwT[k], in_=inwT[k * 128:(k + 1) * 128, :])
                w_xpwT = [const.tile([128, DR + 2 * DS], BF, tag="wxpw", name="wxpw",
                                     bufs=NDT) for _ in range(NDT)]
                for k in range(NDT):
                    nc.scalar.dma_start(out=w_xpwT[k], in_=xpwT[k * 128:(k + 1) * 128, :])
                w_dtwT = const.tile([DR, DI], BF, tag="wdtw", name="wdtw")
                nc.scalar.dma_start(out=w_dtwT, in_=dtwT[:, :])
                w_weffT = [const.tile([128, DM], BF, tag="wow", name="wow", bufs=NDT)
                           for _ in range(NDT)]
                for k in range(NDT):
                    nc.sync.dma_start(out=w_weffT[k],
                                      in_=weffT[k * 128:(k + 1) * 128, :])
                w_convw = [const.tile([128, 4], F32, tag="wconv", name="wconv",
                                      bufs=NDT) for _ in range(NDT)]
                w_convb = [const.tile([128, 1], F32, tag="wconvb", name="wconvb",
                                      bufs=NDT) for _ in range(NDT)]
                w_dtb = [const.tile([128, 1], F32, tag="wdtb", name="wdtb",
                                    bufs=NDT) for _ in range(NDT)]
                w_dvec = [const.tile([128, 1], F32, tag="wdvec", name="wdvec",
                                     bufs=NDT) for _ in range(NDT)]
                for k in range(NDT):
                    sl = slice(k * 128, (k + 1) * 128)
                    nc.gpsimd.dma_start(out=w_convw[k], in_=convw[sl, :])
                    nc.gpsimd.dma_start(out=w_convb[k], in_=convb[sl, :])
                    nc.scalar.dma_start(out=w_dtb[k], in_=dtb[sl, :])
                    nc.sync.dma_start(out=w_dvec[k], in_=dvec[sl, :])

                # --- in-projection u blocks: u -> u2m seg (as raw u), then
                # conv taps read the seg in place, silu overwrites it with u2.
                # u-proj runs CHUNK-major (c outer) so conv-half0 of every
                # d-tile is ready after half the u-proj work; conv halves are
                # emitted between the chunk rounds. x_dbl c0 needs half0 of
                # all four d-tiles, so this shortens the prologue chain.
                HW2 = T // 2

                def _uproj_round(c):
                    cs = slice(c * CH, (c + 1) * CH)
                    for mb in range(NDT):
                        p, kk = mb // 2, mb % 2
                        seg0 = kk * (T + 1)
                        pmm = ps.tile([128, CH], F32, tag="pmm", name="pmm")
                        for k in range(2):
                            nc.tensor.matmul(pmm, w_inwT[k][:, mb * 128:(mb + 1) * 128],
                                             xn[k][:, cs], start=(k == 0), stop=(k == 1))
                        nc.scalar.copy(
                            out=u2m[p][:, seg0 + c * CH:seg0 + (c + 1) * CH],
                            in_=pmm)

                _uproj_round(0)
                _uproj_round(1)

                def _conv_half(d, half):
                    p, kk = d // 2, d % 2
                    seg0 = kk * (T + 1)
                    useg = u2m[p][:, seg0:seg0 + T]
                    # conv as independent tap products per half (DVE
                    # tensor_scalar, 4x) summed with shifts on the PE into a
                    # 2-bank PSUM half (one matmul per 512 sub-chunk); silu
                    # reads PSUM directly.
                    for half in (half,):
                        g0 = half * HW2
                        pcv = pss.tile([128, HW2], F32, tag="pcv", name="pcv",
                                       bufs=2)
                        for i, k in enumerate((3, 2, 1, 0)):   # shift s = 3-k
                            s = 3 - k
                            a = max(s - g0, 0)
                            mk = big.tile([128, HW2], BF, tag="mtap", name="mk",
                                          bufs=2)
                            if k == 3:
                                nc.vector.tensor_scalar(
                                    out=mk, in0=useg[:, g0:g0 + HW2],
                                    scalar1=w_convw[d][:, 3:4],
                                    scalar2=w_convb[d],
                                    op0=OP.mult, op1=OP.add)
                            else:
                                nc.vector.tensor_scalar(
                                    out=mk[:, a:HW2],
                                    in0=u2m[p][:, seg0 + g0 + a - s:
                                               seg0 + g0 + HW2 - s],
                                    scalar1=w_convw[d][:, k:k + 1],
                                    scalar2=None, op0=OP.mult)
                            for q in range(2):
                                lo = max(a, q * CH)
                                hi = (q + 1) * CH
                                nc.tensor.matmul(pcv[:, lo:hi], ident,
                                                 mk[:, lo:hi],
                                                 start=(i == 0), stop=(i == 3))
                        nc.scalar.activation(u2m[p][:, seg0 + g0:seg0 + g0 + HW2],
                                             pcv, AF.Silu, bias=0.0, scale=1.0)

                for d in range(NDT):
                    _conv_half(d, 0)
                _uproj_round(2)
                _uproj_round(3)
                for d in range(NDT):
                    _conv_half(d, 1)

                # --- x_dbl = xpwT.T @ u2 ([48, T]), pair-0 delta (native
                # Softplus activation, no Exp/Ln scratch dance), and the FULL
                # n=0 pipeline fused per chunk: bc bounce, dA, dBu, chunked
                # scans chained via initial=h[:, prev_last], and yp. The first
                # scan piece starts as soon as chunk 0 of the delta chain
                # lands instead of waiting for the full-width tiles.
                dtBC = const.tile([DR + 2 * DS, T], BF, tag="dtbc", name="dtbc")

                def dual_ap(t, c0, w):
                    """[128, 2, w] AP over both mega segments at col c0."""
                    return bass.AP(tensor=t.tensor, offset=t.offset + c0,
                                   ap=[t.ap[0], [T + 1, 2], [1, w]])

                bc0 = nbp.tile([128, 2 * TP], BF, tag="bcn", name="bcn", bufs=3)
                dA0 = nbp.tile([128, MW], BF, tag="dA", name="dA", bufs=2)
                dBu0 = nbp.tile([128, MW], BF, tag="dBu", name="dBu", bufs=1)
                h0 = nbp.tile([128, MW], BF, tag="h", name="h", bufs=1)
                yp0 = nbp.tile([128, MW], BF, tag="yp", name="yp", bufs=4)
                a_0 = float(avals[0])
                for c in range(NCHUNK):
                    cs = slice(c * CH, (c + 1) * CH)
                    pdb = pss.tile([DR + 2 * DS, CH], F32, tag="px", name="pdb")
                    for d in range(NDT):
                        p, kk = d // 2, d % 2
                        seg = slice(kk * (T + 1) + c * CH, kk * (T + 1) + (c + 1) * CH)
                        nc.tensor.matmul(pdb, w_xpwT[d], u2m[p][:, seg],
                                         start=(d == 0), stop=(d == NDT - 1))
                    nc.scalar.copy(out=dtBC[:, cs], in_=pdb)
                    nc.sync.dma_start(out=bcb[:, cs],
                                      in_=dtBC[DR:DR + 2 * DS, cs])
                    for d in (0, 1):
                        kk = d % 2
                        seg = slice(kk * (T + 1) + c * CH,
                                    kk * (T + 1) + (c + 1) * CH)
                        pda = ps.tile([128, CH], F32, tag="pmm", name="pda")
                        nc.tensor.matmul(pda, w_dtwT[:, d * 128:(d + 1) * 128],
                                         dtBC[0:DR, cs], start=True, stop=True)
                        nc.scalar.activation(wdm[0][:, seg], pda, AF.Exp,
                                             bias=w_dtb[d], scale=1.0)

                # softplus tail: Ln(1+exp) per segment, then the chunked n=0
                # pipeline (wdm fill, bc bounce, dA, dBu, chained scans, yp)
                nc.scalar.activation(dlm[0][:, 0:T], wdm[0][:, 0:T],
                                     AF.Ln, bias=1.0, scale=1.0)
                nc.scalar.activation(dlm[0][:, T + 1:MW - 1],
                                     wdm[0][:, T + 1:MW - 1],
                                     AF.Ln, bias=1.0, scale=1.0)
                a_0 = float(avals[0])
                for c in range(NCHUNK):
                    nc.vector.tensor_tensor(out=dual_ap(wdm[0], c * CH, CH),
                                            in0=dual_ap(dlm[0], c * CH, CH),
                                            in1=dual_ap(u2m[0], c * CH, CH),
                                            op=OP.mult)
                    bcout = bass.AP(tensor=bc0.tensor, offset=bc0.offset + c * CH,
                                    ap=[bc0.ap[0], [TP, 2], [1, CH]])
                    bcin = bcb[0:1, c * CH:(c + 1) * CH]
                    nc.gpsimd.dma_start(
                        out=bcout,
                        in_=bass.AP(tensor=bcin.tensor, offset=bcin.offset,
                                    ap=[[0, 128], [DS * TP, 2], [1, CH]]))
                    nc.scalar.activation(dual_ap(dA0, c * CH, CH),
                                         dual_ap(dlm[0], c * CH, CH),
                                         AF.Exp, bias=0.0, scale=a_0)
                    nc.vector.tensor_tensor(
                        out=dual_ap(dBu0, c * CH, CH),
                        in0=dual_ap(wdm[0], c * CH, CH),
                        in1=bass.AP(tensor=bc0.tensor, offset=bc0.offset + c * CH,
                                    ap=[bc0.ap[0], [0, 2], [1, CH]]),
                        op=OP.mult)
                    for kk in range(2):
                        seg = slice(kk * (T + 1) + c * CH,
                                    kk * (T + 1) + (c + 1) * CH)
                        init = 0.0 if c == 0 else h0[:, kk * (T + 1) + c * CH - 1:
                                                     kk * (T + 1) + c * CH]
                        nc.vector.tensor_tensor_scan(h0[:, seg], dA0[:, seg],
                                                     dBu0[:, seg], init,
                                                     op0=OP.mult, op1=OP.add)
                    nc.vector.tensor_tensor(
                        out=dual_ap(yp0, c * CH, CH),
                        in0=dual_ap(h0, c * CH, CH),
                        in1=bass.AP(tensor=bc0.tensor,
                                    offset=bc0.offset + TP + c * CH,
                                    ap=[bc0.ap[0], [0, 2], [1, CH]]),
                        op=OP.mult)

                # --- delta for the remaining d-tiles: Exp scratch + Ln(1+x)
                def _delta_mm(d):
                    p, kk = d // 2, d % 2
                    for c in range(NCHUNK):
                        seg = slice(kk * (T + 1) + c * CH, kk * (T + 1) + (c + 1) * CH)
                        pda = ps.tile([128, CH], F32, tag="pmm", name="pda")
                        nc.tensor.matmul(pda, w_dtwT[:, d * 128:(d + 1) * 128],
                                         dtBC[0:DR, c * CH:(c + 1) * CH],
                                         start=True, stop=True)
                        nc.scalar.activation(wdm[p][:, seg], pda, AF.Exp,
                                             bias=w_dtb[d], scale=1.0)

                def _delta_fin(p):
                    nc.scalar.activation(dlm[p][:, 0:T], wdm[p][:, 0:T],
                                         AF.Ln, bias=1.0, scale=1.0)
                    nc.scalar.activation(dlm[p][:, T + 1:MW - 1],
                                         wdm[p][:, T + 1:MW - 1],
                                         AF.Ln, bias=1.0, scale=1.0)
                    nc.vector.tensor_mul(wdm[p], dlm[p], u2m[p])

                # --- z-projection blocks, right after pair-0's delta chain.
                # Drains go through DVE tensor_scalar (NOT Act) so the Act
                # queue stays clear for the held dA exps, and the prologue
                # PSUM tiles release early (psy needs all 8 banks). All four
                # silus are applied in place during the scan passes (Act has
                # slack there).
                def _zblock(mb):
                    for c in range(NCHUNK):
                        cs = slice(c * CH, (c + 1) * CH)
                        pmm = ps.tile([128, CH], F32, tag="pmm", name="pmm")
                        for k in range(2):
                            nc.tensor.matmul(pmm, w_inwT[k][:, mb * 128:(mb + 1) * 128],
                                             xn[k][:, cs], start=(k == 0), stop=(k == 1))
                        nc.scalar.copy(out=sz[mb - NDT][:, cs], in_=pmm)

                def emit_n_compute(p, n):
                    a_n = float(avals[n])
                    # one combined broadcast per n: B row then C row
                    bc = nbp.tile([128, 2 * TP], BF, tag="bcn", name="bcn",
                                  bufs=3)
                    nc.gpsimd.dma_start(out=bc, in_=_bc2_ap(bcb, n, TP))
                    dA = nbp.tile([128, MW], BF, tag="dA", name="dA", bufs=2)
                    nc.scalar.activation(dA, dlm[p], AF.Exp, bias=0.0, scale=a_n)
                    dBu = nbp.tile([128, MW], BF, tag="dBu", name="dBu", bufs=1)
                    nc.vector.tensor_tensor(out=dBu, in0=wdm[p],
                                            in1=_rep2_ap(bc, TP), op=OP.mult)
                    h = nbp.tile([128, MW], BF, tag="h", name="h", bufs=1)
                    nc.vector.tensor_tensor_scan(h, dA, dBu, 0.0,
                                                 op0=OP.mult, op1=OP.add)
                    # NOTE: do NOT offload these mults to the Pool engine —
                    # Pool shares SBUF ports with DVE and concurrent Pool
                    # ops slow DVE scans ~1.5x (measured 8.7us -> 12.3us).
                    yp = nbp.tile([128, MW], BF, tag="yp", name="yp", bufs=4)
                    nc.vector.tensor_tensor(out=yp, in0=h,
                                            in1=_rep2_ap(bc, TP, TP), op=OP.mult)
                    return yp

                def emit_n_mm(pyac, n, yp):
                    for k in range(2):
                        for c in range(NCHUNK):
                            seg = slice(k * (T + 1) + c * CH,
                                        k * (T + 1) + (c + 1) * CH)
                            nc.tensor.matmul(pyac[k * NCHUNK + c], ident,
                                             yp[:, seg], start=(n == 0),
                                             stop=False)

                # held pass-0 computes: n=0 came from the fused chunk loop;
                # n=1..3 are mega ops. PE accumulations deferred to psy.
                held = [yp0, emit_n_compute(0, 1)]
                for mb in range(NDT, 2 * NDT):
                    _zblock(mb)
                _delta_mm(2)
                _delta_mm(3)
                _delta_fin(1)
                held.append(emit_n_compute(0, 2))
                held.append(emit_n_compute(0, 3))

            # --- selective scan: 2 passes over d-pairs ----------------------
            with tc.tile_pool(name="psy", bufs=8, space="PSUM") as psy:
                def mk_drain(p, pyac):
                    # drain + gate: the u2*D skip is added on the PE via a
                    # diag(D) stationary closing the accumulation group, so
                    # the gate is one TT mult reading PSUM directly (GPSIMD
                    # cannot touch PSUM, so both passes gate on DVE)
                    def _drain():
                        eng = nc.vector
                        for c in range(NCHUNK):
                            cs = slice(c * CH, (c + 1) * CH)
                            for k in range(2):
                                d = 2 * p + k
                                seg = slice(k * (T + 1) + c * CH,
                                            k * (T + 1) + (c + 1) * CH)
                                nc.tensor.matmul(pyac[k * NCHUNK + c], w_ddg[d],
                                                 u2m[p][:, seg], start=False,
                                                 stop=True)
                                eng.tensor_tensor(out=ygc[d][:, cs],
                                                  in0=pyac[k * NCHUNK + c],
                                                  in1=sz[d][:, cs],
                                                  op=OP.mult)
                    return _drain

                pend_drain = None
                for p in range(2):
                    # 8 psum accumulators: (k in pair, chunk) -> [128, 512]
                    pyac = [psy.tile([128, CH], F32, tag="pyac", name="pyac")
                            for _ in range(8)]
                    n0 = 0
                    if p == 0:
                        for n in range(len(held)):
                            emit_n_mm(pyac, n, held[n])
                        n0 = len(held)
                    for n in range(n0, DS):
                        yp = emit_n_compute(p, n)
                        if pend_drain is not None:
                            # pass-0 drain emitted after pass-1's first
                            # compute: its gate TTs queue behind that scan so
                            # DVE never stalls at the pass boundary
                            pend_drain()
                            pend_drain = None
                        emit_n_mm(pyac, n, yp)
                        # deferred in-place z silus, two per pass, spread so
                        # each hides behind a scan's worth of Act slack
                        if p == 0 and n in (6, 8):
                            dz = (n - 6) // 2
                            nc.scalar.activation(sz[dz], sz[dz], AF.Silu,
                                                 bias=0.0, scale=1.0)
                        if p == 1 and n in (2, 3):
                            dz = n  # sz[2] at n==2, sz[3] at n==3
                            nc.scalar.activation(sz[dz], sz[dz], AF.Silu,
                                                 bias=0.0, scale=1.0)
                    pend_drain = mk_drain(p, pyac)
                pend_drain()

            # --- epilogue: fused (fus_w @ out_w) projection, chunked.
            # Act drains PSUM to bf16; output DMAs split across two queues
            # (sync + gpsimd) so the tail DMA drain is not serialized.
            with tc.tile_pool(name="pse", bufs=3, space="PSUM") as pse:
                for c in range(NCHUNK):
                    cs = slice(c * CH, (c + 1) * CH)
                    for mb in range(2):
                        pmf = pse.tile([128, CH], F32, tag="pme", name="pmf")
                        for k in range(NDT):
                            nc.tensor.matmul(pmf, w_weffT[k][:, mb * 128:(mb + 1) * 128],
                                             ygc[k][:, cs], start=(k == 0),
                                             stop=(k == NDT - 1))
                        osb = work.tile([128, CH], BF, tag="osb", name="osb", bufs=2)
                        nc.scalar.copy(out=osb, in_=pmf)
                        h2 = CH // 2
                        nc.sync.dma_start(out=o2[mb * 128:(mb + 1) * 128,
                                                 c * CH:c * CH + h2],
                                          in_=osb[:, 0:h2])
                        nc.gpsimd.dma_start(out=o2[mb * 128:(mb + 1) * 128,
                                                   c * CH + h2:(c + 1) * CH],
                                            in_=osb[:, h2:CH])

    nc.finalize()
    return nc


def _prep_core(xn_b, inp, pfx, direction, fus_w, idh):
    """Host-side input map for one core. xn_b is the pre-normalized x."""
    bf16 = ml_dtypes.bfloat16
    xt = np.ascontiguousarray(xn_b.T)
    if direction:
        xt = np.ascontiguousarray(xt[:, ::-1])
    g = lambda k: np.asarray(inp[pfx + k])
    w_eff = fus_w[:, direction * DM:(direction + 1) * DM].astype(np.float32) @ \
        g("out_w").astype(np.float32)          # [DM, DI]
    m = {
        "xt": xt.astype(bf16),
        "inwT": np.ascontiguousarray(g("in_w").T).astype(bf16),
        "xpwT": np.ascontiguousarray(g("xproj_w").T).astype(bf16),
        "dtwT": np.ascontiguousarray(g("dt_w").T).astype(bf16),
        "weffT": np.ascontiguousarray(w_eff.T).astype(bf16),
        "convw": np.ascontiguousarray(g("conv_w")).astype(np.float32),
        "convb": g("conv_b").reshape(DI, 1).astype(np.float32),
        "dtb": g("dt_b").reshape(DI, 1).astype(np.float32),
        "dvec": g("D").reshape(DI, 1).astype(np.float32),
        "idh": idh,
        "ddg": np.concatenate([np.diag(g("D")[kq * 128:(kq + 1) * 128])
                               for kq in range(NDT)], axis=0).astype(bf16),
    }
    return m


def _run(inputs, trace=False):
    x = np.asarray(inputs["x"], np.float32)
    B = x.shape[0]
    assert x.shape == (4, T, DM), x.shape
    fus_w = np.asarray(inputs["fus_w"], np.float32)
    fus_b = np.asarray(inputs["fus_b"], np.float32)
    norm_w = np.asarray(inputs["norm_w"], np.float32)
    norm_b = np.asarray(inputs["norm_b"], np.float32)
    # layernorm on host (pure input preprocessing, like the transposes)
    mu = x.mean(-1, keepdims=True)
    var = x.var(-1, keepdims=True)
    xnorm = (x - mu) / np.sqrt(var + 1e-5) * norm_w + norm_b
    idh = np.eye(128, dtype=ml_dtypes.bfloat16)

    avals_f = -np.exp(np.asarray(inputs["f_A_log"], np.float32)[0])
    avals_b = -np.exp(np.asarray(inputs["b_A_log"], np.float32)[0])
    assert np.allclose(avals_f, avals_b), "A must match across directions"
    key = avals_f.tobytes()
    if key not in _CACHE:
        _CACHE[key] = _build(avals_f)
    nc = _CACHE[key]

    in_maps = []
    for b in range(B):
        for direction in (0, 1):
            pfx = "b_" if direction else "f_"
            in_maps.append(_prep_core(xnorm[b], inputs, pfx, direction,
                                      fus_w, idh))

    res = run_bass_kernel_spmd(nc, in_maps, list(range(8)), trace=trace)
    out = np.empty((B, T, DM), np.float32)
    for b in range(B):
        of = np.asarray(res.results[2 * b]["o2"], np.float32)
        ob = np.asarray(res.results[2 * b + 1]["o2"], np.float32)[:, ::-1]
        out[b] = (of + ob).T + x[b] + fus_b[None, :]
    return out, res


def kernel(**inputs):
    out, _ = _run(inputs, trace=False)
    return out



# revision 46
# speedup vs baseline: 1.0069x; 1.0069x over previous
"""Bidirectional Mamba block on 8 Trainium2 NeuronCores.

Sharding: 8 cores = 4 batches x 2 directions (fwd/bwd). Each core runs the
full per-(batch, direction) Mamba pipeline on a time-transposed slice
x[b].T (time-flipped for the backward direction), producing its partial
contribution to the fused output projection. Host sums fwd+bwd partials,
adds the residual and fusion bias.

v2 layout: [d (partitions), t (free)], selective scan restructured:
  - n-loop runs in 2 passes over d-PAIRS using mega tiles [128, 4098]
    (two 2048-column d-tile segments + poisoned boundary columns where
    delta=1e9 -> dA=exp(A*1e9)=0 and u2=0 -> dBu=0, so one
    tensor_tensor_scan instruction covers both segments with a clean
    state reset).
  - B/C broadcast tiles are [128, 2049]; the mega elementwise mults read
    them twice via a stride-0 middle AP dim (keeps DVE 2x mode).
  - y = sum_n h_n*C_n accumulated on the TENSOR engine: per n, 8
    identity matmuls [128x128x512] accumulate yp slices into 8 PSUM
    banks (2 d-tiles x 4 t-chunks). DVE no longer does the adds.
  - gate fused with the PSUM drain (diag(D) close matmul + one TT).

v3 scheduling (the scan window is at the DVE floor of ~13.3us/iter =
dBu TT 2.3 + scan 8.7 + yp TT 2.3; HW rates: TT 2x=0.56ns/el,
tensor_scalar 4x=0.30ns/el, scan 2.12ns/col, Act 0.9ns/el):
  - fus_w @ out_w folded on the host -> single-GEMM epilogue.
  - weight DMAs split across the sync/Act/gpsimd descriptor queues so
    u-proj does not queue behind the x loads.
  - n=0 pipeline fused per chunk into the x_dbl loop with CHAINED
    chunk scans (initial=h[:, prev_last]) - the first scan piece
    issues ~15us before the full-width delta would be ready.
  - z-projection emitted between held n1 and the pair-1 delta; PSUM
    drains via Act so the prologue PSUM frees before psy needs banks.
  - pass-0 drain emitted after pass-1's first compute.
  - PSUM->SBUF drains on Act (scalar.copy); output DMAs on 2 queues.
CAUTION: engine times vary ~20% across device allocations (clock
bins); compare runs via the mega-scan duration (8.69us fast bin).
"""

import numpy as np
import ml_dtypes

import concourse.bass as bass
import concourse.bacc as bacc
import concourse.tile as tile
from concourse import mybir
from concourse.bass_utils import run_bass_kernel_spmd

T = 2048
TP = T + 1          # broadcast tile width (padded)
MW = 2 * T + 2      # mega width: [0:T) seg A, T poison, [T+1:2T+1) seg B, 2T+1 poison
DM = 256      # d_model
DI = 512      # d_inner
DS = 16       # d_state
DR = 16       # dt_rank
NCHUNK = 4    # matmul moving-dim chunks of 512
CH = T // NCHUNK
NDT = DI // 128  # 4 d-tiles of 128 partitions

BF = mybir.dt.bfloat16
F32 = mybir.dt.float32
AF = mybir.ActivationFunctionType
OP = mybir.AluOpType

_CACHE = {}


def _bcast_ap(dram_handle, row, col0, width):
    """AP reading dram[row, col0:col0+width] broadcast across 128 partitions."""
    base = dram_handle[row:row + 1, col0:col0 + width]
    return bass.AP(tensor=base.tensor, offset=base.offset,
                   ap=[[0, 128], [1, width]])


def _rep2_ap(tile_, width, col0=0):
    """Free-replicated read of tile_[:, col0:col0+width] twice."""
    return bass.AP(tensor=tile_.tensor, offset=tile_.offset + col0,
                   ap=[tile_.ap[0], [0, 2], [1, width]])


def _bc2_ap(dram_handle, row, width):
    """Broadcast rows `row` and `row+DS` of bcb as one [128, 2*width] read."""
    base = dram_handle[row:row + 1, 0:width]
    return bass.AP(tensor=base.tensor, offset=base.offset,
                   ap=[[0, 128], [DS * width, 2], [1, width]])


def _build(avals):
    nc = bacc.Bacc()

    # --- I/O ---------------------------------------------------------------
    xt = nc.declare_dram_parameter("xt", [DM, T], BF, isOutput=False)
    inwT = nc.declare_dram_parameter("inwT", [DM, 2 * DI], BF, isOutput=False)
    xpwT = nc.declare_dram_parameter("xpwT", [DI, DR + 2 * DS], BF, isOutput=False)
    dtwT = nc.declare_dram_parameter("dtwT", [DR, DI], BF, isOutput=False)
    # weffT = (fus_w_half @ out_w).T — the fusion matmul folded into the
    # output projection on the host, so the epilogue is a single GEMM
    weffT = nc.declare_dram_parameter("weffT", [DI, DM], BF, isOutput=False)
    convw = nc.declare_dram_parameter("convw", [DI, 4], F32, isOutput=False)
    convb = nc.declare_dram_parameter("convb", [DI, 1], F32, isOutput=False)
    dtb = nc.declare_dram_parameter("dtb", [DI, 1], F32, isOutput=False)
    dvec = nc.declare_dram_parameter("dvec", [DI, 1], F32, isOutput=False)
    idh = nc.declare_dram_parameter("idh", [128, 128], BF, isOutput=False)
    ddg = nc.declare_dram_parameter("ddg", [NDT * 128, 128], BF, isOutput=False)
    o2 = nc.declare_dram_parameter("o2", [DM, T], BF, isOutput=True)

    # DRAM scratch for partition-broadcast bounces
    bcb = nc.dram_tensor("bcb", [2 * DS, TP], BF)   # B rows 0..15, C rows 16..31

    with tile.TileContext(nc) as tc:
        with (
            tc.tile_pool(name="const", bufs=1) as const,
            tc.tile_pool(name="big", bufs=2) as big,
            tc.tile_pool(name="pers", bufs=4) as pers,
            tc.tile_pool(name="work", bufs=2) as work,
            tc.tile_pool(name="nb_", bufs=2) as nbp,
        ):
            # --- load x (chunked across DMA queues) -------------------------
            xn = [big.tile([128, T], BF, tag="xn", name="xn", bufs=2)
                  for _ in range(2)]
            for k in range(2):
                for c in range(NCHUNK):
                    cs = slice(c * CH, (c + 1) * CH)
                    nc.sync.dma_start(out=xn[k][:, cs],
                                      in_=xt[k * 128:(k + 1) * 128, cs])


            ident = const.tile([128, 128], BF, tag="ident", name="ident")
            nc.gpsimd.dma_start(out=ident, in_=idh[:, :])
            w_ddg = [const.tile([128, 128], BF, tag="wddg", name="wddg",
                                bufs=NDT) for _ in range(NDT)]
            for kq in range(NDT):
                nc.sync.dma_start(out=w_ddg[kq], in_=ddg[kq * 128:(kq + 1) * 128, :])
            # zero the padded column of the B/C bounce buffer so the
            # broadcast reads a finite value at the mega poison column
            zrow = const.tile([2 * DS, 1], BF, tag="zrow", name="zrow")
            nc.vector.memset(zrow, 0.0)
            nc.sync.dma_start(out=bcb[:, T:TP], in_=zrow)
            # persistent mega tiles (2 d-pairs)
            u2m = [pers.tile([128, MW], BF, tag="u2m", name="u2m", bufs=2)
                   for _ in range(2)]
            dlm = [pers.tile([128, MW], BF, tag="dlm", name="dlm", bufs=2)
                   for _ in range(2)]
            wdm = [pers.tile([128, MW], BF, tag="wdm", name="wdm", bufs=2)
                   for _ in range(2)]
            sz = [pers.tile([128, T], BF, tag="sz", name="sz") for _ in range(NDT)]
            ygc = [pers.tile([128, T], BF, tag="ygc", name="ygc")
                   for _ in range(NDT)]
            # poison columns: delta=1e9, u2=0, wdm=0 at cols T and 2T+1
            # (wdm poisons are memset because the chunked pair-0 fill never
            # touches them, and stale NaN*0 = NaN would break the scan reset)
            for p in range(2):
                nc.vector.memset(dlm[p][:, T:T + 1], 1e9)
                nc.vector.memset(dlm[p][:, MW - 1:MW], 1e9)
                nc.vector.memset(u2m[p][:, T:T + 1], 0.0)
                nc.vector.memset(u2m[p][:, MW - 1:MW], 0.0)
                nc.vector.memset(wdm[p][:, T:T + 1], 0.0)
                nc.vector.memset(wdm[p][:, MW - 1:MW], 0.0)

            def mseg(p, k):
                """Segment slice of mega tile for d-tile index (2*p + k)."""
                return slice(k * (T + 1), k * (T + 1) + T)

            with tc.tile_pool(name="ps", bufs=2, space="PSUM") as ps, \
                 tc.tile_pool(name="pss", bufs=2, space="PSUM") as pss:
                # --- weights/constants ------------------------------------------
                w_inwT = [const.tile([128, 2 * DI], BF, tag="winw", name="winw",
                                     bufs=2) for _ in range(2)]
                for k in range(2):
                    nc.gpsimd.dma_start(out=w_inwT[k], in_=inwT[k * 128:(k + 1) * 128, :])
                w_xpwT = [const.tile([128, DR + 2 * DS], BF, tag="wxpw", name="wxpw",
                                     bufs=NDT) for _ in range(NDT)]
                for k in range(NDT):
                    nc.scalar.dma_start(out=w_xpwT[k], in_=xpwT[k * 128:(k + 1) * 128, :])
                w_dtwT = const.tile([DR, DI], BF, tag="wdtw", name="wdtw")
                nc.scalar.dma_start(out=w_dtwT, in_=dtwT[:, :])
                w_weffT = [const.tile([128, DM], BF, tag="wow", name="wow", bufs=NDT)
                           for _ in range(NDT)]
                for k in range(NDT):
                    nc.sync.dma_start(out=w_weffT[k],
                                      in_=weffT[k * 128:(k + 1) * 128, :])
                w_convw = [const.tile([128, 4], F32, tag="wconv", name="wconv",
                                      bufs=NDT) for _ in range(NDT)]
                w_convb = [const.tile([128, 1], F32, tag="wconvb", name="wconvb",
                                      bufs=NDT) for _ in range(NDT)]
                w_dtb = [const.tile([128, 1], F32, tag="wdtb", name="wdtb",
                                    bufs=NDT) for _ in range(NDT)]
                w_dvec = [const.tile([128, 1], F32, tag="wdvec", name="wdvec",
                                     bufs=NDT) for _ in range(NDT)]
                for k in range(NDT):
                    sl = slice(k * 128, (k + 1) * 128)
                    nc.gpsimd.dma_start(out=w_convw[k], in_=convw[sl, :])
                    nc.gpsimd.dma_start(out=w_convb[k], in_=convb[sl, :])
                    nc.scalar.dma_start(out=w_dtb[k], in_=dtb[sl, :])
                    nc.sync.dma_start(out=w_dvec[k], in_=dvec[sl, :])

                # --- in-projection u blocks: u -> u2m seg (as raw u), then
                # conv taps read the seg in place, silu overwrites it with u2.
                # u-proj runs CHUNK-major (c outer) so conv-half0 of every
                # d-tile is ready after half the u-proj work; conv halves are
                # emitted between the chunk rounds. x_dbl c0 needs half0 of
                # all four d-tiles, so this shortens the prologue chain.
                HW2 = T // 2

                def _uproj_round(c):
                    cs = slice(c * CH, (c + 1) * CH)
                    for mb in range(NDT):
                        p, kk = mb // 2, mb % 2
                        seg0 = kk * (T + 1)
                        pmm = ps.tile([128, CH], F32, tag="pmm", name="pmm")
                        for k in range(2):
                            nc.tensor.matmul(pmm, w_inwT[k][:, mb * 128:(mb + 1) * 128],
                                             xn[k][:, cs], start=(k == 0), stop=(k == 1))
                        nc.scalar.copy(
                            out=u2m[p][:, seg0 + c * CH:seg0 + (c + 1) * CH],
                            in_=pmm)

                _uproj_round(0)
                _uproj_round(1)

                def _conv_half(d, half):
                    p, kk = d // 2, d % 2
                    seg0 = kk * (T + 1)
                    useg = u2m[p][:, seg0:seg0 + T]
                    # conv as independent tap products per half (DVE
                    # tensor_scalar, 4x) summed with shifts on the PE into a
                    # 2-bank PSUM half (one matmul per 512 sub-chunk); silu
                    # reads PSUM directly.
                    for half in (half,):
                        g0 = half * HW2
                        pcv = pss.tile([128, HW2], F32, tag="pcv", name="pcv",
                                       bufs=2)
                        for i, k in enumerate((3, 2, 1, 0)):   # shift s = 3-k
                            s = 3 - k
                            a = max(s - g0, 0)
                            mk = big.tile([128, HW2], BF, tag="mtap", name="mk",
                                          bufs=2)
                            if k == 3:
                                nc.vector.tensor_scalar(
                                    out=mk, in0=useg[:, g0:g0 + HW2],
                                    scalar1=w_convw[d][:, 3:4],
                                    scalar2=w_convb[d],
                                    op0=OP.mult, op1=OP.add)
                            else:
                                nc.vector.tensor_scalar(
                                    out=mk[:, a:HW2],
                                    in0=u2m[p][:, seg0 + g0 + a - s:
                                               seg0 + g0 + HW2 - s],
                                    scalar1=w_convw[d][:, k:k + 1],
                                    scalar2=None, op0=OP.mult)
                            for q in range(2):
                                lo = max(a, q * CH)
                                hi = (q + 1) * CH
                                nc.tensor.matmul(pcv[:, lo:hi], ident,
                                                 mk[:, lo:hi],
                                                 start=(i == 0), stop=(i == 3))
                        nc.scalar.activation(u2m[p][:, seg0 + g0:seg0 + g0 + HW2],
                                             pcv, AF.Silu, bias=0.0, scale=1.0)

                for d in range(NDT):
                    _conv_half(d, 0)
                _uproj_round(2)
                _uproj_round(3)
                for d in range(NDT):
                    _conv_half(d, 1)

                # --- x_dbl = xpwT.T @ u2 ([48, T]), pair-0 delta (native
                # Softplus activation, no Exp/Ln scratch dance), and the FULL
                # n=0 pipeline fused per chunk: bc bounce, dA, dBu, chunked
                # scans chained via initial=h[:, prev_last], and yp. The first
                # scan piece starts as soon as chunk 0 of the delta chain
                # lands instead of waiting for the full-width tiles.
                dtBC = const.tile([DR + 2 * DS, T], BF, tag="dtbc", name="dtbc")

                def dual_ap(t, c0, w):
                    """[128, 2, w] AP over both mega segments at col c0."""
                    return bass.AP(tensor=t.tensor, offset=t.offset + c0,
                                   ap=[t.ap[0], [T + 1, 2], [1, w]])

                bc0 = nbp.tile([128, 2 * TP], BF, tag="bcn", name="bcn", bufs=3)
                dA0 = nbp.tile([128, MW], BF, tag="dA", name="dA", bufs=2)
                dBu0 = nbp.tile([128, MW], BF, tag="dBu", name="dBu", bufs=1)
                h0 = nbp.tile([128, MW], BF, tag="h", name="h", bufs=1)
                yp0 = nbp.tile([128, MW], BF, tag="yp", name="yp", bufs=4)
                a_0 = float(avals[0])
                for c in range(NCHUNK):
                    cs = slice(c * CH, (c + 1) * CH)
                    pdb = pss.tile([DR + 2 * DS, CH], F32, tag="px", name="pdb")
                    for d in range(NDT):
                        p, kk = d // 2, d % 2
                        seg = slice(kk * (T + 1) + c * CH, kk * (T + 1) + (c + 1) * CH)
                        nc.tensor.matmul(pdb, w_xpwT[d], u2m[p][:, seg],
                                         start=(d == 0), stop=(d == NDT - 1))
                    nc.scalar.copy(out=dtBC[:, cs], in_=pdb)
                    nc.sync.dma_start(out=bcb[:, cs],
                                      in_=dtBC[DR:DR + 2 * DS, cs])
                    for d in (0, 1):
                        kk = d % 2
                        seg = slice(kk * (T + 1) + c * CH,
                                    kk * (T + 1) + (c + 1) * CH)
                        pda = ps.tile([128, CH], F32, tag="pmm", name="pda")
                        nc.tensor.matmul(pda, w_dtwT[:, d * 128:(d + 1) * 128],
                                         dtBC[0:DR, cs], start=True, stop=True)
                        nc.scalar.activation(wdm[0][:, seg], pda, AF.Exp,
                                             bias=w_dtb[d], scale=1.0)

                # softplus tail: Ln(1+exp) per segment, then the chunked n=0
                # pipeline (wdm fill, bc bounce, dA, dBu, chained scans, yp)
                nc.scalar.activation(dlm[0][:, 0:T], wdm[0][:, 0:T],
                                     AF.Ln, bias=1.0, scale=1.0)
                nc.scalar.activation(dlm[0][:, T + 1:MW - 1],
                                     wdm[0][:, T + 1:MW - 1],
                                     AF.Ln, bias=1.0, scale=1.0)
                a_0 = float(avals[0])
                for c in range(NCHUNK):
                    nc.vector.tensor_tensor(out=dual_ap(wdm[0], c * CH, CH),
                                            in0=dual_ap(dlm[0], c * CH, CH),
                                            in1=dual_ap(u2m[0], c * CH, CH),
                                            op=OP.mult)
                    bcout = bass.AP(tensor=bc0.tensor, offset=bc0.offset + c * CH,
                                    ap=[bc0.ap[0], [TP, 2], [1, CH]])
                    bcin = bcb[0:1, c * CH:(c + 1) * CH]
                    nc.gpsimd.dma_start(
                        out=bcout,
                        in_=bass.AP(tensor=bcin.tensor, offset=bcin.offset,
                                    ap=[[0, 128], [DS * TP, 2], [1, CH]]))
                    nc.scalar.activation(dual_ap(dA0, c * CH, CH),
                                         dual_ap(dlm[0], c * CH, CH),
                                         AF.Exp, bias=0.0, scale=a_0)
                    nc.vector.tensor_tensor(
                        out=dual_ap(dBu0, c * CH, CH),
                        in0=dual_ap(wdm[0], c * CH, CH),
                        in1=bass.AP(tensor=bc0.tensor, offset=bc0.offset + c * CH,
                                    ap=[bc0.ap[0], [0, 2], [1, CH]]),
                        op=OP.mult)
                    for kk in range(2):
                        seg = slice(kk * (T + 1) + c * CH,
                                    kk * (T + 1) + (c + 1) * CH)
                        init = 0.0 if c == 0 else h0[:, kk * (T + 1) + c * CH - 1:
                                                     kk * (T + 1) + c * CH]
                        nc.vector.tensor_tensor_scan(h0[:, seg], dA0[:, seg],
                                                     dBu0[:, seg], init,
                                                     op0=OP.mult, op1=OP.add)
                    nc.vector.tensor_tensor(
                        out=dual_ap(yp0, c * CH, CH),
                        in0=dual_ap(h0, c * CH, CH),
                        in1=bass.AP(tensor=bc0.tensor,
                                    offset=bc0.offset + TP + c * CH,
                                    ap=[bc0.ap[0], [0, 2], [1, CH]]),
                        op=OP.mult)

                # --- delta for the remaining d-tiles: Exp scratch + Ln(1+x)
                def _delta_mm(d):
                    p, kk = d // 2, d % 2
                    for c in range(NCHUNK):
                        seg = slice(kk * (T + 1) + c * CH, kk * (T + 1) + (c + 1) * CH)
                        pda = ps.tile([128, CH], F32, tag="pmm", name="pda")
                        nc.tensor.matmul(pda, w_dtwT[:, d * 128:(d + 1) * 128],
                                         dtBC[0:DR, c * CH:(c + 1) * CH],
                                         start=True, stop=True)
                        nc.scalar.activation(wdm[p][:, seg], pda, AF.Exp,
                                             bias=w_dtb[d], scale=1.0)

                def _delta_fin(p):
                    nc.scalar.activation(dlm[p][:, 0:T], wdm[p][:, 0:T],
                                         AF.Ln, bias=1.0, scale=1.0)
                    nc.scalar.activation(dlm[p][:, T + 1:MW - 1],
                                         wdm[p][:, T + 1:MW - 1],
                                         AF.Ln, bias=1.0, scale=1.0)
                    nc.vector.tensor_mul(wdm[p], dlm[p], u2m[p])

                # --- z-projection blocks, right after pair-0's delta chain.
                # Drains go through DVE tensor_scalar (NOT Act) so the Act
                # queue stays clear for the held dA exps, and the prologue
                # PSUM tiles release early (psy needs all 8 banks). All four
                # silus are applied in place during the scan passes (Act has
                # slack there).
                def _zblock(mb):
                    for c in range(NCHUNK):
                        cs = slice(c * CH, (c + 1) * CH)
                        pmm = ps.tile([128, CH], F32, tag="pmm", name="pmm")
                        for k in range(2):
                            nc.tensor.matmul(pmm, w_inwT[k][:, mb * 128:(mb + 1) * 128],
                                             xn[k][:, cs], start=(k == 0), stop=(k == 1))
                        nc.scalar.copy(out=sz[mb - NDT][:, cs], in_=pmm)

                def emit_n_compute(p, n):
                    a_n = float(avals[n])
                    # one combined broadcast per n: B row then C row
                    bc = nbp.tile([128, 2 * TP], BF, tag="bcn", name="bcn",
                                  bufs=3)
                    nc.gpsimd.dma_start(out=bc, in_=_bc2_ap(bcb, n, TP))
                    dA = nbp.tile([128, MW], BF, tag="dA", name="dA", bufs=2)
                    nc.scalar.activation(dA, dlm[p], AF.Exp, bias=0.0, scale=a_n)
                    dBu = nbp.tile([128, MW], BF, tag="dBu", name="dBu", bufs=1)
                    nc.vector.tensor_tensor(out=dBu, in0=wdm[p],
                                            in1=_rep2_ap(bc, TP), op=OP.mult)
                    h = nbp.tile([128, MW], BF, tag="h", name="h", bufs=1)
                    nc.vector.tensor_tensor_scan(h, dA, dBu, 0.0,
                                                 op0=OP.mult, op1=OP.add)
                    # NOTE: do NOT offload these mults to the Pool engine —
                    # Pool shares SBUF ports with DVE and concurrent Pool
                    # ops slow DVE scans ~1.5x (measured 8.7us -> 12.3us).
                    yp = nbp.tile([128, MW], BF, tag="yp", name="yp", bufs=4)
                    nc.vector.tensor_tensor(out=yp, in0=h,
                                            in1=_rep2_ap(bc, TP, TP), op=OP.mult)
                    return yp

                def emit_n_mm(pyac, n, yp):
                    # the diag(D) skip-mm OPENS each group (emit_d_open), so
                    # n=15 closes it and the gate fires without a close-mm
                    for k in range(2):
                        for c in range(NCHUNK):
                            seg = slice(k * (T + 1) + c * CH,
                                        k * (T + 1) + (c + 1) * CH)
                            nc.tensor.matmul(pyac[k * NCHUNK + c], ident,
                                             yp[:, seg], start=False,
                                             stop=(n == DS - 1))

                def emit_d_open(p, pyac):
                    # u2*D skip term as the accumulation-group STARTER: it
                    # only needs u2m, which is ready long before drain time
                    for c in range(NCHUNK):
                        for k in range(2):
                            d = 2 * p + k
                            seg = slice(k * (T + 1) + c * CH,
                                        k * (T + 1) + (c + 1) * CH)
                            nc.tensor.matmul(pyac[k * NCHUNK + c], w_ddg[d],
                                             u2m[p][:, seg], start=True,
                                             stop=False)

                # held pass-0 computes: n=0 came from the fused chunk loop;
                # n=1..3 are mega ops. PE accumulations deferred to psy.
                held = [yp0, emit_n_compute(0, 1)]
                for mb in range(NDT, 2 * NDT):
                    _zblock(mb)
                _delta_mm(2)
                _delta_mm(3)
                _delta_fin(1)
                held.append(emit_n_compute(0, 2))
                held.append(emit_n_compute(0, 3))

            # --- selective scan: 2 passes over d-pairs ----------------------
            with tc.tile_pool(name="psy", bufs=8, space="PSUM") as psy:
                def mk_drain(p, pyac):
                    # drain = just the gate TT reading PSUM (the diag(D) skip
                    # opened the group; n=15's accum mm closed it). GPSIMD
                    # cannot touch PSUM, so both passes gate on DVE.
                    def _drain():
                        for c in range(NCHUNK):
                            cs = slice(c * CH, (c + 1) * CH)
                            for k in range(2):
                                d = 2 * p + k
                                nc.vector.tensor_tensor(
                                    out=ygc[d][:, cs],
                                    in0=pyac[k * NCHUNK + c],
                                    in1=sz[d][:, cs], op=OP.mult)
                    return _drain

                pend_drain = None
                for p in range(2):
                    # 8 psum accumulators: (k in pair, chunk) -> [128, 512]
                    pyac = [psy.tile([128, CH], F32, tag="pyac", name="pyac")
                            for _ in range(8)]
                    emit_d_open(p, pyac)
                    n0 = 0
                    if p == 0:
                        for n in range(len(held)):
                            emit_n_mm(pyac, n, held[n])
                        n0 = len(held)
                    for n in range(n0, DS):
                        yp = emit_n_compute(p, n)
                        if pend_drain is not None:
                            # pass-0 drain emitted after pass-1's first
                            # compute: its gate TTs queue behind that scan so
                            # DVE never stalls at the pass boundary
                            pend_drain()
                            pend_drain = None
                        emit_n_mm(pyac, n, yp)
                        # deferred in-place z silus, two per pass, spread so
                        # each hides behind a scan's worth of Act slack
                        if p == 0 and n in (6, 8):
                            dz = (n - 6) // 2
                            nc.scalar.activation(sz[dz], sz[dz], AF.Silu,
                                                 bias=0.0, scale=1.0)
                        if p == 1 and n in (2, 3):
                            dz = n  # sz[2] at n==2, sz[3] at n==3
                            nc.scalar.activation(sz[dz], sz[dz], AF.Silu,
                                                 bias=0.0, scale=1.0)
                    pend_drain = mk_drain(p, pyac)
                pend_drain()

            # --- epilogue: fused (fus_w @ out_w) projection, chunked.
            # Act drains PSUM to bf16; output DMAs split across two queues
            # (sync + gpsimd) so the tail DMA drain is not serialized.
            with tc.tile_pool(name="pse", bufs=3, space="PSUM") as pse:
                for c in range(NCHUNK):
                    cs = slice(c * CH, (c + 1) * CH)
                    for mb in range(2):
                        pmf = pse.tile([128, CH], F32, tag="pme", name="pmf")
                        for k in range(NDT):
                            nc.tensor.matmul(pmf, w_weffT[k][:, mb * 128:(mb + 1) * 128],
                                             ygc[k][:, cs], start=(k == 0),
                                             stop=(k == NDT - 1))
                        osb = work.tile([128, CH], BF, tag="osb", name="osb", bufs=2)
                        nc.scalar.copy(out=osb, in_=pmf)
                        h2 = CH // 2
                        nc.sync.dma_start(out=o2[mb * 128:(mb + 1) * 128,
                                                 c * CH:c * CH + h2],
                                          in_=osb[:, 0:h2])
                        nc.gpsimd.dma_start(out=o2[mb * 128:(mb + 1) * 128,
                                                   c * CH + h2:(c + 1) * CH],
                                            in_=osb[:, h2:CH])

    nc.finalize()
    return nc


def _prep_core(xn_b, inp, pfx, direction, fus_w, idh):
    """Host-side input map for one core. xn_b is the pre-normalized x."""
    bf16 = ml_dtypes.bfloat16
    xt = np.ascontiguousarray(xn_b.T)
    if direction:
        xt = np.ascontiguousarray(xt[:, ::-1])
    g = lambda k: np.asarray(inp[pfx + k])
    w_eff = fus_w[:, direction * DM:(direction + 1) * DM].astype(np.float32) @ \
        g("out_w").astype(np.float32)          # [DM, DI]
    m = {
        "xt": xt.astype(bf16),
        "inwT": np.ascontiguousarray(g("in_w").T).astype(bf16),
        "xpwT": np.ascontiguousarray(g("xproj_w").T).astype(bf16),
        "dtwT": np.ascontiguousarray(g("dt_w").T).astype(bf16),
        "weffT": np.ascontiguousarray(w_eff.T).astype(bf16),
        "convw": np.ascontiguousarray(g("conv_w")).astype(np.float32),
        "convb": g("conv_b").reshape(DI, 1).astype(np.float32),
        "dtb": g("dt_b").reshape(DI, 1).astype(np.float32),
        "dvec": g("D").reshape(DI, 1).astype(np.float32),
        "idh": idh,
        "ddg": np.concatenate([np.diag(g("D")[kq * 128:(kq + 1) * 128])
                               for kq in range(NDT)], axis=0).astype(bf16),
    }
    return m


def _run(inputs, trace=False):
    x = np.asarray(inputs["x"], np.float32)
    B = x.shape[0]
    assert x.shape == (4, T, DM), x.shape
    fus_w = np.asarray(inputs["fus_w"], np.float32)
    fus_b = np.asarray(inputs["fus_b"], np.float32)
    norm_w = np.asarray(inputs["norm_w"], np.float32)
    norm_b = np.asarray(inputs["norm_b"], np.float32)
    # layernorm on host (pure input preprocessing, like the transposes)
    mu = x.mean(-1, keepdims=True)
    var = x.var(-1, keepdims=True)
    xnorm = (x - mu) / np.sqrt(var + 1e-5) * norm_w + norm_b
    idh = np.eye(128, dtype=ml_dtypes.bfloat16)

    avals_f = -np.exp(np.asarray(inputs["f_A_log"], np.float32)[0])
    avals_b = -np.exp(np.asarray(inputs["b_A_log"], np.float32)[0])
    assert np.allclose(avals_f, avals_b), "A must match across directions"
    key = avals_f.tobytes()
    if key not in _CACHE:
        _CACHE[key] = _build(avals_f)
    nc = _CACHE[key]

    in_maps = []
    for b in range(B):
        for direction in (0, 1):
            pfx = "b_" if direction else "f_"
            in_maps.append(_prep_core(xnorm[b], inputs, pfx, direction,
                                      fus_w, idh))

    res = run_bass_kernel_spmd(nc, in_maps, list(range(8)), trace=trace)
    out = np.empty((B, T, DM), np.float32)
    for b in range(B):
        of = np.asarray(res.results[2 * b]["o2"], np.float32)
        ob = np.asarray(res.results[2 * b + 1]["o2"], np.float32)[:, ::-1]
        out[b] = (of + ob).T + x[b] + fus_b[None, :]
    return out, res


def kernel(**inputs):
    out, _ = _run(inputs, trace=False)
    return out



# revision 47
# speedup vs baseline: 1.0138x; 1.0068x over previous
"""Bidirectional Mamba block on 8 Trainium2 NeuronCores.

Sharding: 8 cores = 4 batches x 2 directions (fwd/bwd). Each core runs the
full per-(batch, direction) Mamba pipeline on a time-transposed slice
x[b].T (time-flipped for the backward direction), producing its partial
contribution to the fused output projection. Host sums fwd+bwd partials,
adds the residual and fusion bias.

v2 layout: [d (partitions), t (free)], selective scan restructured:
  - n-loop runs in 2 passes over d-PAIRS using mega tiles [128, 4098]
    (two 2048-column d-tile segments + poisoned boundary columns where
    delta=1e9 -> dA=exp(A*1e9)=0 and u2=0 -> dBu=0, so one
    tensor_tensor_scan instruction covers both segments with a clean
    state reset).
  - B/C broadcast tiles are [128, 2049]; the mega elementwise mults read
    them twice via a stride-0 middle AP dim (keeps DVE 2x mode).
  - y = sum_n h_n*C_n accumulated on the TENSOR engine: per n, 8
    identity matmuls [128x128x512] accumulate yp slices into 8 PSUM
    banks (2 d-tiles x 4 t-chunks). DVE no longer does the adds.
  - gate fused with the PSUM drain (diag(D) close matmul + one TT).

v3 scheduling (the scan window is at the DVE floor of ~13.3us/iter =
dBu TT 2.3 + scan 8.7 + yp TT 2.3; HW rates: TT 2x=0.56ns/el,
tensor_scalar 4x=0.30ns/el, scan 2.12ns/col, Act 0.9ns/el):
  - fus_w @ out_w folded on the host -> single-GEMM epilogue.
  - weight DMAs split across the sync/Act/gpsimd descriptor queues so
    u-proj does not queue behind the x loads.
  - n=0 pipeline fused per chunk into the x_dbl loop with CHAINED
    chunk scans (initial=h[:, prev_last]) - the first scan piece
    issues ~15us before the full-width delta would be ready.
  - z-projection emitted between held n1 and the pair-1 delta; PSUM
    drains via Act so the prologue PSUM frees before psy needs banks.
  - pass-0 drain emitted after pass-1's first compute.
  - PSUM->SBUF drains on Act (scalar.copy); output DMAs on 2 queues.
CAUTION: engine times vary ~20% across device allocations (clock
bins); compare runs via the mega-scan duration (8.69us fast bin).
"""

import numpy as np
import ml_dtypes

import concourse.bass as bass
import concourse.bacc as bacc
import concourse.tile as tile
from concourse import mybir
from concourse.bass_utils import run_bass_kernel_spmd

T = 2048
TP = T + 1          # broadcast tile width (padded)
MW = 2 * T + 2      # mega width: [0:T) seg A, T poison, [T+1:2T+1) seg B, 2T+1 poison
DM = 256      # d_model
DI = 512      # d_inner
DS = 16       # d_state
DR = 16       # dt_rank
NCHUNK = 4    # matmul moving-dim chunks of 512
CH = T // NCHUNK
NDT = DI // 128  # 4 d-tiles of 128 partitions

BF = mybir.dt.bfloat16
F32 = mybir.dt.float32
AF = mybir.ActivationFunctionType
OP = mybir.AluOpType

_CACHE = {}


def _bcast_ap(dram_handle, row, col0, width):
    """AP reading dram[row, col0:col0+width] broadcast across 128 partitions."""
    base = dram_handle[row:row + 1, col0:col0 + width]
    return bass.AP(tensor=base.tensor, offset=base.offset,
                   ap=[[0, 128], [1, width]])


def _rep2_ap(tile_, width, col0=0):
    """Free-replicated read of tile_[:, col0:col0+width] twice."""
    return bass.AP(tensor=tile_.tensor, offset=tile_.offset + col0,
                   ap=[tile_.ap[0], [0, 2], [1, width]])


def _bc2_ap(dram_handle, row, width):
    """Broadcast rows `row` and `row+DS` of bcb as one [128, 2*width] read."""
    base = dram_handle[row:row + 1, 0:width]
    return bass.AP(tensor=base.tensor, offset=base.offset,
                   ap=[[0, 128], [DS * width, 2], [1, width]])


def _build(avals):
    nc = bacc.Bacc()

    # --- I/O ---------------------------------------------------------------
    xt = nc.declare_dram_parameter("xt", [DM, T], BF, isOutput=False)
    inwT = nc.declare_dram_parameter("inwT", [DM, 2 * DI], BF, isOutput=False)
    xpwT = nc.declare_dram_parameter("xpwT", [DI, DR + 2 * DS], BF, isOutput=False)
    dtwT = nc.declare_dram_parameter("dtwT", [DR, DI], BF, isOutput=False)
    # weffT = (fus_w_half @ out_w).T — the fusion matmul folded into the
    # output projection on the host, so the epilogue is a single GEMM
    weffT = nc.declare_dram_parameter("weffT", [DI, DM], BF, isOutput=False)
    convw = nc.declare_dram_parameter("convw", [DI, 4], F32, isOutput=False)
    convb = nc.declare_dram_parameter("convb", [DI, 1], F32, isOutput=False)
    dtb = nc.declare_dram_parameter("dtb", [DI, 1], F32, isOutput=False)
    dvec = nc.declare_dram_parameter("dvec", [DI, 1], F32, isOutput=False)
    idh = nc.declare_dram_parameter("idh", [128, 128], BF, isOutput=False)
    ddg = nc.declare_dram_parameter("ddg", [NDT * 128, 128], BF, isOutput=False)
    o2 = nc.declare_dram_parameter("o2", [DM, T], BF, isOutput=True)

    # DRAM scratch for partition-broadcast bounces
    bcb = nc.dram_tensor("bcb", [2 * DS, TP], BF)   # B rows 0..15, C rows 16..31

    with tile.TileContext(nc) as tc:
        with (
            tc.tile_pool(name="const", bufs=1) as const,
            tc.tile_pool(name="big", bufs=2) as big,
            tc.tile_pool(name="pers", bufs=4) as pers,
            tc.tile_pool(name="work", bufs=2) as work,
            tc.tile_pool(name="nb_", bufs=2) as nbp,
        ):
            # --- load x (chunked across DMA queues) -------------------------
            xn = [big.tile([128, T], BF, tag="xn", name="xn", bufs=2)
                  for _ in range(2)]
            for k in range(2):
                for c in range(NCHUNK):
                    cs = slice(c * CH, (c + 1) * CH)
                    nc.sync.dma_start(out=xn[k][:, cs],
                                      in_=xt[k * 128:(k + 1) * 128, cs])


            ident = const.tile([128, 128], BF, tag="ident", name="ident")
            nc.gpsimd.dma_start(out=ident, in_=idh[:, :])
            w_ddg = [const.tile([128, 128], BF, tag="wddg", name="wddg",
                                bufs=NDT) for _ in range(NDT)]
            for kq in range(NDT):
                nc.sync.dma_start(out=w_ddg[kq], in_=ddg[kq * 128:(kq + 1) * 128, :])
            # zero the padded column of the B/C bounce buffer so the
            # broadcast reads a finite value at the mega poison column
            zrow = const.tile([2 * DS, 1], BF, tag="zrow", name="zrow")
            nc.vector.memset(zrow, 0.0)
            nc.sync.dma_start(out=bcb[:, T:TP], in_=zrow)
            # persistent mega tiles (2 d-pairs)
            u2m = [pers.tile([128, MW], BF, tag="u2m", name="u2m", bufs=2)
                   for _ in range(2)]
            dlm = [pers.tile([128, MW], BF, tag="dlm", name="dlm", bufs=2)
                   for _ in range(2)]
            wdm = [pers.tile([128, MW], BF, tag="wdm", name="wdm", bufs=2)
                   for _ in range(2)]
            sz = [pers.tile([128, T], BF, tag="sz", name="sz") for _ in range(NDT)]
            ygc = [pers.tile([128, T], BF, tag="ygc", name="ygc")
                   for _ in range(NDT)]
            # poison columns: delta=1e9, u2=0, wdm=0 at cols T and 2T+1
            # (wdm poisons are memset because the chunked pair-0 fill never
            # touches them, and stale NaN*0 = NaN would break the scan reset)
            for p in range(2):
                nc.vector.memset(dlm[p][:, T:T + 1], 1e9)
                nc.vector.memset(dlm[p][:, MW - 1:MW], 1e9)
                nc.vector.memset(u2m[p][:, T:T + 1], 0.0)
                nc.vector.memset(u2m[p][:, MW - 1:MW], 0.0)
                nc.vector.memset(wdm[p][:, T:T + 1], 0.0)
                nc.vector.memset(wdm[p][:, MW - 1:MW], 0.0)

            def mseg(p, k):
                """Segment slice of mega tile for d-tile index (2*p + k)."""
                return slice(k * (T + 1), k * (T + 1) + T)

            with tc.tile_pool(name="ps", bufs=2, space="PSUM") as ps, \
                 tc.tile_pool(name="pss", bufs=2, space="PSUM") as pss:
                # --- weights/constants ------------------------------------------
                w_inwT = [const.tile([128, 2 * DI], BF, tag="winw", name="winw",
                                     bufs=2) for _ in range(2)]
                for k in range(2):
                    nc.gpsimd.dma_start(out=w_inwT[k], in_=inwT[k * 128:(k + 1) * 128, :])
                w_xpwT = [const.tile([128, DR + 2 * DS], BF, tag="wxpw", name="wxpw",
                                     bufs=NDT) for _ in range(NDT)]
                for k in range(NDT):
                    nc.scalar.dma_start(out=w_xpwT[k], in_=xpwT[k * 128:(k + 1) * 128, :])
                w_dtwT = const.tile([DR, DI], BF, tag="wdtw", name="wdtw")
                nc.scalar.dma_start(out=w_dtwT, in_=dtwT[:, :])
                w_weffT = [const.tile([128, DM], BF, tag="wow", name="wow", bufs=NDT)
                           for _ in range(NDT)]
                for k in range(NDT):
                    nc.sync.dma_start(out=w_weffT[k],
                                      in_=weffT[k * 128:(k + 1) * 128, :])
                w_convw = [const.tile([128, 4], F32, tag="wconv", name="wconv",
                                      bufs=NDT) for _ in range(NDT)]
                w_convb = [const.tile([128, 1], F32, tag="wconvb", name="wconvb",
                                      bufs=NDT) for _ in range(NDT)]
                w_dtb = [const.tile([128, 1], F32, tag="wdtb", name="wdtb",
                                    bufs=NDT) for _ in range(NDT)]
                w_dvec = [const.tile([128, 1], F32, tag="wdvec", name="wdvec",
                                     bufs=NDT) for _ in range(NDT)]
                for k in range(NDT):
                    sl = slice(k * 128, (k + 1) * 128)
                    nc.gpsimd.dma_start(out=w_convw[k], in_=convw[sl, :])
                    nc.gpsimd.dma_start(out=w_convb[k], in_=convb[sl, :])
                    nc.scalar.dma_start(out=w_dtb[k], in_=dtb[sl, :])
                    nc.sync.dma_start(out=w_dvec[k], in_=dvec[sl, :])

                # --- in-projection u blocks: u -> u2m seg (as raw u), then
                # conv taps read the seg in place, silu overwrites it with u2.
                # u-proj runs CHUNK-major (c outer) so conv-half0 of every
                # d-tile is ready after half the u-proj work; conv halves are
                # emitted between the chunk rounds. x_dbl c0 needs half0 of
                # all four d-tiles, so this shortens the prologue chain.
                HW2 = T // 2

                def _uproj_round(c):
                    cs = slice(c * CH, (c + 1) * CH)
                    for mb in range(NDT):
                        p, kk = mb // 2, mb % 2
                        seg0 = kk * (T + 1)
                        pmm = ps.tile([128, CH], F32, tag="pmm", name="pmm")
                        for k in range(2):
                            nc.tensor.matmul(pmm, w_inwT[k][:, mb * 128:(mb + 1) * 128],
                                             xn[k][:, cs], start=(k == 0), stop=(k == 1))
                        nc.scalar.copy(
                            out=u2m[p][:, seg0 + c * CH:seg0 + (c + 1) * CH],
                            in_=pmm)

                _uproj_round(0)
                _uproj_round(1)

                def _conv_half(d, half):
                    p, kk = d // 2, d % 2
                    seg0 = kk * (T + 1)
                    useg = u2m[p][:, seg0:seg0 + T]
                    # conv as independent tap products per half (DVE
                    # tensor_scalar, 4x) summed with shifts on the PE into a
                    # 2-bank PSUM half (one matmul per 512 sub-chunk); silu
                    # reads PSUM directly.
                    for half in (half,):
                        g0 = half * HW2
                        pcv = pss.tile([128, HW2], F32, tag="pcv", name="pcv",
                                       bufs=2)
                        for i, k in enumerate((3, 2, 1, 0)):   # shift s = 3-k
                            s = 3 - k
                            a = max(s - g0, 0)
                            mk = big.tile([128, HW2], BF, tag="mtap", name="mk",
                                          bufs=2)
                            if k == 3:
                                nc.vector.tensor_scalar(
                                    out=mk, in0=useg[:, g0:g0 + HW2],
                                    scalar1=w_convw[d][:, 3:4],
                                    scalar2=w_convb[d],
                                    op0=OP.mult, op1=OP.add)
                            else:
                                nc.vector.tensor_scalar(
                                    out=mk[:, a:HW2],
                                    in0=u2m[p][:, seg0 + g0 + a - s:
                                               seg0 + g0 + HW2 - s],
                                    scalar1=w_convw[d][:, k:k + 1],
                                    scalar2=None, op0=OP.mult)
                            for q in range(2):
                                lo = max(a, q * CH)
                                hi = (q + 1) * CH
                                nc.tensor.matmul(pcv[:, lo:hi], ident,
                                                 mk[:, lo:hi],
                                                 start=(i == 0), stop=(i == 3))
                        nc.scalar.activation(u2m[p][:, seg0 + g0:seg0 + g0 + HW2],
                                             pcv, AF.Silu, bias=0.0, scale=1.0)

                for d in range(NDT):
                    _conv_half(d, 0)
                _uproj_round(2)
                _uproj_round(3)
                for d in range(NDT):
                    _conv_half(d, 1)

                # --- x_dbl = xpwT.T @ u2 ([48, T]), pair-0 delta (native
                # Softplus activation, no Exp/Ln scratch dance), and the FULL
                # n=0 pipeline fused per chunk: bc bounce, dA, dBu, chunked
                # scans chained via initial=h[:, prev_last], and yp. The first
                # scan piece starts as soon as chunk 0 of the delta chain
                # lands instead of waiting for the full-width tiles.
                dtBC = const.tile([DR + 2 * DS, T], BF, tag="dtbc", name="dtbc")

                def dual_ap(t, c0, w):
                    """[128, 2, w] AP over both mega segments at col c0."""
                    return bass.AP(tensor=t.tensor, offset=t.offset + c0,
                                   ap=[t.ap[0], [T + 1, 2], [1, w]])

                bc0 = nbp.tile([128, 2 * TP], BF, tag="bcn", name="bcn", bufs=3)
                dA0 = nbp.tile([128, MW], BF, tag="dA", name="dA", bufs=2)
                dBu0 = nbp.tile([128, MW], BF, tag="dBu", name="dBu", bufs=1)
                h0 = nbp.tile([128, MW], BF, tag="h", name="h", bufs=1)
                yp0 = nbp.tile([128, MW], BF, tag="yp", name="yp", bufs=4)
                a_0 = float(avals[0])
                for c in range(NCHUNK):
                    cs = slice(c * CH, (c + 1) * CH)
                    pdb = pss.tile([DR + 2 * DS, CH], F32, tag="px", name="pdb")
                    for d in range(NDT):
                        p, kk = d // 2, d % 2
                        seg = slice(kk * (T + 1) + c * CH, kk * (T + 1) + (c + 1) * CH)
                        nc.tensor.matmul(pdb, w_xpwT[d], u2m[p][:, seg],
                                         start=(d == 0), stop=(d == NDT - 1))
                    nc.scalar.copy(out=dtBC[:, cs], in_=pdb)
                    nc.sync.dma_start(out=bcb[:, cs],
                                      in_=dtBC[DR:DR + 2 * DS, cs])
                    for d in (0, 1):
                        kk = d % 2
                        seg = slice(kk * (T + 1) + c * CH,
                                    kk * (T + 1) + (c + 1) * CH)
                        pda = ps.tile([128, CH], F32, tag="pmm", name="pda")
                        nc.tensor.matmul(pda, w_dtwT[:, d * 128:(d + 1) * 128],
                                         dtBC[0:DR, cs], start=True, stop=True)
                        nc.scalar.activation(wdm[0][:, seg], pda, AF.Exp,
                                             bias=w_dtb[d], scale=1.0)

                # softplus tail: Ln(1+exp) per segment, then the chunked n=0
                # pipeline (wdm fill, bc bounce, dA, dBu, chained scans, yp)
                nc.scalar.activation(dlm[0][:, 0:T], wdm[0][:, 0:T],
                                     AF.Ln, bias=1.0, scale=1.0)
                nc.scalar.activation(dlm[0][:, T + 1:MW - 1],
                                     wdm[0][:, T + 1:MW - 1],
                                     AF.Ln, bias=1.0, scale=1.0)
                a_0 = float(avals[0])
                for c in range(NCHUNK):
                    nc.vector.tensor_tensor(out=dual_ap(wdm[0], c * CH, CH),
                                            in0=dual_ap(dlm[0], c * CH, CH),
                                            in1=dual_ap(u2m[0], c * CH, CH),
                                            op=OP.mult)
                    bcout = bass.AP(tensor=bc0.tensor, offset=bc0.offset + c * CH,
                                    ap=[bc0.ap[0], [TP, 2], [1, CH]])
                    bcin = bcb[0:1, c * CH:(c + 1) * CH]
                    nc.gpsimd.dma_start(
                        out=bcout,
                        in_=bass.AP(tensor=bcin.tensor, offset=bcin.offset,
                                    ap=[[0, 128], [DS * TP, 2], [1, CH]]))
                    nc.scalar.activation(dual_ap(dA0, c * CH, CH),
                                         dual_ap(dlm[0], c * CH, CH),
                                         AF.Exp, bias=0.0, scale=a_0)
                    nc.vector.tensor_tensor(
                        out=dual_ap(dBu0, c * CH, CH),
                        in0=dual_ap(wdm[0], c * CH, CH),
                        in1=bass.AP(tensor=bc0.tensor, offset=bc0.offset + c * CH,
                                    ap=[bc0.ap[0], [0, 2], [1, CH]]),
                        op=OP.mult)
                    for kk in range(2):
                        seg = slice(kk * (T + 1) + c * CH,
                                    kk * (T + 1) + (c + 1) * CH)
                        init = 0.0 if c == 0 else h0[:, kk * (T + 1) + c * CH - 1:
                                                     kk * (T + 1) + c * CH]
                        nc.vector.tensor_tensor_scan(h0[:, seg], dA0[:, seg],
                                                     dBu0[:, seg], init,
                                                     op0=OP.mult, op1=OP.add)
                    nc.vector.tensor_tensor(
                        out=dual_ap(yp0, c * CH, CH),
                        in0=dual_ap(h0, c * CH, CH),
                        in1=bass.AP(tensor=bc0.tensor,
                                    offset=bc0.offset + TP + c * CH,
                                    ap=[bc0.ap[0], [0, 2], [1, CH]]),
                        op=OP.mult)

                # --- delta for the remaining d-tiles: Exp scratch + Ln(1+x)
                def _delta_mm(d):
                    p, kk = d // 2, d % 2
                    for c in range(NCHUNK):
                        seg = slice(kk * (T + 1) + c * CH, kk * (T + 1) + (c + 1) * CH)
                        pda = ps.tile([128, CH], F32, tag="pmm", name="pda")
                        nc.tensor.matmul(pda, w_dtwT[:, d * 128:(d + 1) * 128],
                                         dtBC[0:DR, c * CH:(c + 1) * CH],
                                         start=True, stop=True)
                        nc.scalar.activation(wdm[p][:, seg], pda, AF.Exp,
                                             bias=w_dtb[d], scale=1.0)

                def _delta_fin(p):
                    nc.scalar.activation(dlm[p][:, 0:T], wdm[p][:, 0:T],
                                         AF.Ln, bias=1.0, scale=1.0)
                    nc.scalar.activation(dlm[p][:, T + 1:MW - 1],
                                         wdm[p][:, T + 1:MW - 1],
                                         AF.Ln, bias=1.0, scale=1.0)
                    nc.vector.tensor_mul(wdm[p], dlm[p], u2m[p])

                # --- z-projection blocks, right after pair-0's delta chain.
                # Drains go through DVE tensor_scalar (NOT Act) so the Act
                # queue stays clear for the held dA exps, and the prologue
                # PSUM tiles release early (psy needs all 8 banks). All four
                # silus are applied in place during the scan passes (Act has
                # slack there).
                def _zblock(mb):
                    for c in range(NCHUNK):
                        cs = slice(c * CH, (c + 1) * CH)
                        pmm = ps.tile([128, CH], F32, tag="pmm", name="pmm")
                        for k in range(2):
                            nc.tensor.matmul(pmm, w_inwT[k][:, mb * 128:(mb + 1) * 128],
                                             xn[k][:, cs], start=(k == 0), stop=(k == 1))
                        nc.scalar.copy(out=sz[mb - NDT][:, cs], in_=pmm)

                def emit_n_compute(p, n):
                    a_n = float(avals[n])
                    # one combined broadcast per n: B row then C row
                    bc = nbp.tile([128, 2 * TP], BF, tag="bcn", name="bcn",
                                  bufs=3)
                    nc.gpsimd.dma_start(out=bc, in_=_bc2_ap(bcb, n, TP))
                    dA = nbp.tile([128, MW], BF, tag="dA", name="dA", bufs=2)
                    nc.scalar.activation(dA, dlm[p], AF.Exp, bias=0.0, scale=a_n)
                    dBu = nbp.tile([128, MW], BF, tag="dBu", name="dBu", bufs=1)
                    nc.vector.tensor_tensor(out=dBu, in0=wdm[p],
                                            in1=_rep2_ap(bc, TP), op=OP.mult)
                    h = nbp.tile([128, MW], BF, tag="h", name="h", bufs=1)
                    nc.vector.tensor_tensor_scan(h, dA, dBu, 0.0,
                                                 op0=OP.mult, op1=OP.add)
                    # NOTE: do NOT offload these mults to the Pool engine —
                    # Pool shares SBUF ports with DVE and concurrent Pool
                    # ops slow DVE scans ~1.5x (measured 8.7us -> 12.3us).
                    yp = nbp.tile([128, MW], BF, tag="yp", name="yp", bufs=4)
                    nc.vector.tensor_tensor(out=yp, in0=h,
                                            in1=_rep2_ap(bc, TP, TP), op=OP.mult)
                    return yp

                def emit_n_mm(pyac, n, yp):
                    # the diag(D) skip-mm OPENS each group (emit_d_open), so
                    # n=15 closes it and the gate fires without a close-mm
                    for k in range(2):
                        for c in range(NCHUNK):
                            seg = slice(k * (T + 1) + c * CH,
                                        k * (T + 1) + (c + 1) * CH)
                            nc.tensor.matmul(pyac[k * NCHUNK + c], ident,
                                             yp[:, seg], start=False,
                                             stop=(n == DS - 1))

                def emit_d_open(p, pyac):
                    # u2*D skip term as the accumulation-group STARTER: it
                    # only needs u2m, which is ready long before drain time
                    for c in range(NCHUNK):
                        for k in range(2):
                            d = 2 * p + k
                            seg = slice(k * (T + 1) + c * CH,
                                        k * (T + 1) + (c + 1) * CH)
                            nc.tensor.matmul(pyac[k * NCHUNK + c], w_ddg[d],
                                             u2m[p][:, seg], start=True,
                                             stop=False)

                # held pass-0 computes: n=0 came from the fused chunk loop;
                # n=1..3 are mega ops. PE accumulations deferred to psy.
                held = [yp0, emit_n_compute(0, 1)]
                for mb in range(NDT, 2 * NDT):
                    _zblock(mb)
                _delta_mm(2)
                _delta_mm(3)
                _delta_fin(1)
                held.append(emit_n_compute(0, 2))
                held.append(emit_n_compute(0, 3))

            # --- selective scan: 2 passes over d-pairs ----------------------
            with tc.tile_pool(name="psy", bufs=8, space="PSUM") as psy:
                def mk_drain(p, pyac):
                    # drain = gate TT (the diag(D) skip opened the group;
                    # n=15's accum mm closed it). Pass 0: Act copies PSUM ->
                    # ygc (releases the banks for pass 1 without DVE work at
                    # the transition), then an in-place SBUF-side gate TT at
                    # 2x. Pass 1 gates straight from PSUM (shortest tail
                    # chain). GPSIMD cannot touch PSUM.
                    def _drain():
                        for c in range(NCHUNK):
                            cs = slice(c * CH, (c + 1) * CH)
                            for k in range(2):
                                d = 2 * p + k
                                if p == 0:
                                    nc.scalar.copy(out=ygc[d][:, cs],
                                                   in_=pyac[k * NCHUNK + c])
                                    nc.vector.tensor_tensor(
                                        out=ygc[d][:, cs],
                                        in0=ygc[d][:, cs],
                                        in1=sz[d][:, cs], op=OP.mult)
                                else:
                                    nc.vector.tensor_tensor(
                                        out=ygc[d][:, cs],
                                        in0=pyac[k * NCHUNK + c],
                                        in1=sz[d][:, cs], op=OP.mult)
                    return _drain

                pend_drain = None
                for p in range(2):
                    # 8 psum accumulators: (k in pair, chunk) -> [128, 512]
                    pyac = [psy.tile([128, CH], F32, tag="pyac", name="pyac")
                            for _ in range(8)]
                    emit_d_open(p, pyac)
                    n0 = 0
                    if p == 0:
                        for n in range(len(held)):
                            emit_n_mm(pyac, n, held[n])
                        n0 = len(held)
                    for n in range(n0, DS):
                        yp = emit_n_compute(p, n)
                        if pend_drain is not None:
                            # pass-0 drain emitted after pass-1's first
                            # compute: its gate TTs queue behind that scan so
                            # DVE never stalls at the pass boundary
                            pend_drain()
                            pend_drain = None
                        emit_n_mm(pyac, n, yp)
                        # deferred in-place z silus, two per pass, spread so
                        # each hides behind a scan's worth of Act slack
                        if p == 0 and n in (6, 8):
                            dz = (n - 6) // 2
                            nc.scalar.activation(sz[dz], sz[dz], AF.Silu,
                                                 bias=0.0, scale=1.0)
                        if p == 1 and n in (2, 3):
                            dz = n  # sz[2] at n==2, sz[3] at n==3
                            nc.scalar.activation(sz[dz], sz[dz], AF.Silu,
                                                 bias=0.0, scale=1.0)
                    pend_drain = mk_drain(p, pyac)
                pend_drain()

            # --- epilogue: fused (fus_w @ out_w) projection, chunked.
            # Act drains PSUM to bf16; output DMAs split across two queues
            # (sync + gpsimd) so the tail DMA drain is not serialized.
            with tc.tile_pool(name="pse", bufs=3, space="PSUM") as pse:
                for c in range(NCHUNK):
                    cs = slice(c * CH, (c + 1) * CH)
                    for mb in range(2):
                        pmf = pse.tile([128, CH], F32, tag="pme", name="pmf")
                        for k in range(NDT):
                            nc.tensor.matmul(pmf, w_weffT[k][:, mb * 128:(mb + 1) * 128],
                                             ygc[k][:, cs], start=(k == 0),
                                             stop=(k == NDT - 1))
                        osb = work.tile([128, CH], BF, tag="osb", name="osb", bufs=2)
                        nc.scalar.copy(out=osb, in_=pmf)
                        h2 = CH // 2
                        nc.sync.dma_start(out=o2[mb * 128:(mb + 1) * 128,
                                                 c * CH:c * CH + h2],
                                          in_=osb[:, 0:h2])
                        nc.gpsimd.dma_start(out=o2[mb * 128:(mb + 1) * 128,
                                                   c * CH + h2:(c + 1) * CH],
                                            in_=osb[:, h2:CH])

    nc.finalize()
    return nc


def _prep_core(xn_b, inp, pfx, direction, fus_w, idh):
    """Host-side input map for one core. xn_b is the pre-normalized x."""
    bf16 = ml_dtypes.bfloat16
    xt = np.ascontiguousarray(xn_b.T)
    if direction:
        xt = np.ascontiguousarray(xt[:, ::-1])
    g = lambda k: np.asarray(inp[pfx + k])
    w_eff = fus_w[:, direction * DM:(direction + 1) * DM].astype(np.float32) @ \
        g("out_w").astype(np.float32)          # [DM, DI]
    m = {
        "xt": xt.astype(bf16),
        "inwT": np.ascontiguousarray(g("in_w").T).astype(bf16),
        "xpwT": np.ascontiguousarray(g("xproj_w").T).astype(bf16),
        "dtwT": np.ascontiguousarray(g("dt_w").T).astype(bf16),
        "weffT": np.ascontiguousarray(w_eff.T).astype(bf16),
        "convw": np.ascontiguousarray(g("conv_w")).astype(np.float32),
        "convb": g("conv_b").reshape(DI, 1).astype(np.float32),
        "dtb": g("dt_b").reshape(DI, 1).astype(np.float32),
        "dvec": g("D").reshape(DI, 1).astype(np.float32),
        "idh": idh,
        "ddg": np.concatenate([np.diag(g("D")[kq * 128:(kq + 1) * 128])
                               for kq in range(NDT)], axis=0).astype(bf16),
    }
    return m


def _run(inputs, trace=False):
    x = np.asarray(inputs["x"], np.float32)
    B = x.shape[0]
    assert x.shape == (4, T, DM), x.shape
    fus_w = np.asarray(inputs["fus_w"], np.float32)
    fus_b = np.asarray(inputs["fus_b"], np.float32)
    norm_w = np.asarray(inputs["norm_w"], np.float32)
    norm_b = np.asarray(inputs["norm_b"], np.float32)
    # layernorm on host (pure input preprocessing, like the transposes)
    mu = x.mean(-1, keepdims=True)
    var = x.var(-1, keepdims=True)
    xnorm = (x - mu) / np.sqrt(var + 1e-5) * norm_w + norm_b
    idh = np.eye(128, dtype=ml_dtypes.bfloat16)

    avals_f = -np.exp(np.asarray(inputs["f_A_log"], np.float32)[0])
    avals_b = -np.exp(np.asarray(inputs["b_A_log"], np.float32)[0])
    assert np.allclose(avals_f, avals_b), "A must match across directions"
    key = avals_f.tobytes()
    if key not in _CACHE:
        _CACHE[key] = _build(avals_f)
    nc = _CACHE[key]

    in_maps = []
    for b in range(B):
        for direction in (0, 1):
            pfx = "b_" if direction else "f_"
            in_maps.append(_prep_core(xnorm[b], inputs, pfx, direction,
                                      fus_w, idh))

    res = run_bass_kernel_spmd(nc, in_maps, list(range(8)), trace=trace)
    out = np.empty((B, T, DM), np.float32)
    for b in range(B):
        of = np.asarray(res.results[2 * b]["o2"], np.float32)
        ob = np.asarray(res.results[2 * b + 1]["o2"], np.float32)[:, ::-1]
        out[b] = (of + ob).T + x[b] + fus_b[None, :]
    return out, res


def kernel(**inputs):
    out, _ = _run(inputs, trace=False)
    return out



# revision 49
# speedup vs baseline: 1.0154x; 1.0016x over previous
"""Bidirectional Mamba block on 8 Trainium2 NeuronCores.

Sharding: 8 cores = 4 batches x 2 directions (fwd/bwd). Each core runs the
full per-(batch, direction) Mamba pipeline on a time-transposed slice
x[b].T (time-flipped for the backward direction), producing its partial
contribution to the fused output projection. Host sums fwd+bwd partials,
adds the residual and fusion bias.

v2 layout: [d (partitions), t (free)], selective scan restructured:
  - n-loop runs in 2 passes over d-PAIRS using mega tiles [128, 4098]
    (two 2048-column d-tile segments + poisoned boundary columns where
    delta=1e9 -> dA=exp(A*1e9)=0 and u2=0 -> dBu=0, so one
    tensor_tensor_scan instruction covers both segments with a clean
    state reset).
  - B/C broadcast tiles are [128, 2049]; the mega elementwise mults read
    them twice via a stride-0 middle AP dim (keeps DVE 2x mode).
  - y = sum_n h_n*C_n accumulated on the TENSOR engine: per n, 8
    identity matmuls [128x128x512] accumulate yp slices into 8 PSUM
    banks (2 d-tiles x 4 t-chunks). DVE no longer does the adds.
  - gate fused with the PSUM drain (diag(D) close matmul + one TT).

v3 scheduling (the scan window is at the DVE floor of ~13.3us/iter =
dBu TT 2.3 + scan 8.7 + yp TT 2.3; HW rates: TT 2x=0.56ns/el,
tensor_scalar 4x=0.30ns/el, scan 2.12ns/col, Act 0.9ns/el):
  - fus_w @ out_w folded on the host -> single-GEMM epilogue.
  - weight DMAs split across the sync/Act/gpsimd descriptor queues so
    u-proj does not queue behind the x loads.
  - n=0 pipeline fused per chunk into the x_dbl loop with CHAINED
    chunk scans (initial=h[:, prev_last]) - the first scan piece
    issues ~15us before the full-width delta would be ready.
  - z-projection emitted between held n1 and the pair-1 delta; PSUM
    drains via Act so the prologue PSUM frees before psy needs banks.
  - pass-0 drain emitted after pass-1's first compute.
  - PSUM->SBUF drains on Act (scalar.copy); output DMAs on 2 queues.
CAUTION: engine times vary ~20% across device allocations (clock
bins); compare runs via the mega-scan duration (8.69us fast bin).
"""

import numpy as np
import ml_dtypes

import concourse.bass as bass
import concourse.bacc as bacc
import concourse.tile as tile
from concourse import mybir
from concourse.bass_utils import run_bass_kernel_spmd

T = 2048
TP = T + 1          # broadcast tile width (padded)
MW = 2 * T + 2      # mega width: [0:T) seg A, T poison, [T+1:2T+1) seg B, 2T+1 poison
DM = 256      # d_model
DI = 512      # d_inner
DS = 16       # d_state
DR = 16       # dt_rank
NCHUNK = 4    # matmul moving-dim chunks of 512
CH = T // NCHUNK
NDT = DI // 128  # 4 d-tiles of 128 partitions

BF = mybir.dt.bfloat16
F32 = mybir.dt.float32
AF = mybir.ActivationFunctionType
OP = mybir.AluOpType

_CACHE = {}


def _bcast_ap(dram_handle, row, col0, width):
    """AP reading dram[row, col0:col0+width] broadcast across 128 partitions."""
    base = dram_handle[row:row + 1, col0:col0 + width]
    return bass.AP(tensor=base.tensor, offset=base.offset,
                   ap=[[0, 128], [1, width]])


def _rep2_ap(tile_, width, col0=0):
    """Free-replicated read of tile_[:, col0:col0+width] twice."""
    return bass.AP(tensor=tile_.tensor, offset=tile_.offset + col0,
                   ap=[tile_.ap[0], [0, 2], [1, width]])


def _bc2_ap(dram_handle, row, width):
    """Broadcast rows `row` and `row+DS` of bcb as one [128, 2*width] read."""
    base = dram_handle[row:row + 1, 0:width]
    return bass.AP(tensor=base.tensor, offset=base.offset,
                   ap=[[0, 128], [DS * width, 2], [1, width]])


def _build(avals):
    nc = bacc.Bacc()

    # --- I/O ---------------------------------------------------------------
    xt = nc.declare_dram_parameter("xt", [DM, T], BF, isOutput=False)
    inwT = nc.declare_dram_parameter("inwT", [DM, 2 * DI], BF, isOutput=False)
    xpwT = nc.declare_dram_parameter("xpwT", [DI, DR + 2 * DS], BF, isOutput=False)
    dtwT = nc.declare_dram_parameter("dtwT", [DR, DI], BF, isOutput=False)
    # weffT = (fus_w_half @ out_w).T — the fusion matmul folded into the
    # output projection on the host, so the epilogue is a single GEMM
    weffT = nc.declare_dram_parameter("weffT", [DI, DM], BF, isOutput=False)
    convw = nc.declare_dram_parameter("convw", [DI, 4], F32, isOutput=False)
    convb = nc.declare_dram_parameter("convb", [DI, 1], F32, isOutput=False)
    dtb = nc.declare_dram_parameter("dtb", [DI, 1], F32, isOutput=False)
    dvec = nc.declare_dram_parameter("dvec", [DI, 1], F32, isOutput=False)
    idh = nc.declare_dram_parameter("idh", [128, 128], BF, isOutput=False)
    ddg = nc.declare_dram_parameter("ddg", [NDT * 128, 128], BF, isOutput=False)
    o2 = nc.declare_dram_parameter("o2", [DM, T], BF, isOutput=True)

    # DRAM scratch for partition-broadcast bounces
    bcb = nc.dram_tensor("bcb", [2 * DS, TP], BF)   # B rows 0..15, C rows 16..31

    with tile.TileContext(nc) as tc:
        with (
            tc.tile_pool(name="const", bufs=1) as const,
            tc.tile_pool(name="big", bufs=2) as big,
            tc.tile_pool(name="pers", bufs=4) as pers,
            tc.tile_pool(name="work", bufs=2) as work,
            tc.tile_pool(name="nb_", bufs=2) as nbp,
        ):
            # --- load x (chunked across DMA queues) -------------------------
            xn = [big.tile([128, T], BF, tag="xn", name="xn", bufs=2)
                  for _ in range(2)]
            for k in range(2):
                for c in range(NCHUNK):
                    cs = slice(c * CH, (c + 1) * CH)
                    nc.sync.dma_start(out=xn[k][:, cs],
                                      in_=xt[k * 128:(k + 1) * 128, cs])


            ident = const.tile([128, 128], BF, tag="ident", name="ident")
            nc.gpsimd.dma_start(out=ident, in_=idh[:, :])
            w_ddg = [const.tile([128, 128], BF, tag="wddg", name="wddg",
                                bufs=NDT) for _ in range(NDT)]
            for kq in range(NDT):
                nc.sync.dma_start(out=w_ddg[kq], in_=ddg[kq * 128:(kq + 1) * 128, :])
            # zero the padded column of the B/C bounce buffer so the
            # broadcast reads a finite value at the mega poison column
            zrow = const.tile([2 * DS, 1], BF, tag="zrow", name="zrow")
            nc.vector.memset(zrow, 0.0)
            nc.sync.dma_start(out=bcb[:, T:TP], in_=zrow)
            # persistent mega tiles (2 d-pairs)
            u2m = [pers.tile([128, MW], BF, tag="u2m", name="u2m", bufs=2)
                   for _ in range(2)]
            dlm = [pers.tile([128, MW], BF, tag="dlm", name="dlm", bufs=2)
                   for _ in range(2)]
            wdm = [pers.tile([128, MW], BF, tag="wdm", name="wdm", bufs=2)
                   for _ in range(2)]
            sz = [pers.tile([128, T], BF, tag="sz", name="sz") for _ in range(NDT)]
            ygc = [pers.tile([128, T], BF, tag="ygc", name="ygc")
                   for _ in range(NDT)]
            # poison columns: delta=1e9, u2=0, wdm=0 at cols T and 2T+1
            # (wdm poisons are memset because the chunked pair-0 fill never
            # touches them, and stale NaN*0 = NaN would break the scan reset)
            for p in range(2):
                nc.vector.memset(dlm[p][:, T:T + 1], 1e9)
                nc.vector.memset(dlm[p][:, MW - 1:MW], 1e9)
                nc.vector.memset(u2m[p][:, T:T + 1], 0.0)
                nc.vector.memset(u2m[p][:, MW - 1:MW], 0.0)
                nc.vector.memset(wdm[p][:, T:T + 1], 0.0)
                nc.vector.memset(wdm[p][:, MW - 1:MW], 0.0)

            def mseg(p, k):
                """Segment slice of mega tile for d-tile index (2*p + k)."""
                return slice(k * (T + 1), k * (T + 1) + T)

            with tc.tile_pool(name="ps", bufs=2, space="PSUM") as ps, \
                 tc.tile_pool(name="pss", bufs=2, space="PSUM") as pss:
                # --- weights/constants ------------------------------------------
                w_inwT = [const.tile([128, 2 * DI], BF, tag="winw", name="winw",
                                     bufs=2) for _ in range(2)]
                for k in range(2):
                    nc.gpsimd.dma_start(out=w_inwT[k], in_=inwT[k * 128:(k + 1) * 128, :])
                w_xpwT = [const.tile([128, DR + 2 * DS], BF, tag="wxpw", name="wxpw",
                                     bufs=NDT) for _ in range(NDT)]
                for k in range(NDT):
                    nc.scalar.dma_start(out=w_xpwT[k], in_=xpwT[k * 128:(k + 1) * 128, :])
                w_dtwT = const.tile([DR, DI], BF, tag="wdtw", name="wdtw")
                nc.scalar.dma_start(out=w_dtwT, in_=dtwT[:, :])
                w_weffT = [const.tile([128, DM], BF, tag="wow", name="wow", bufs=NDT)
                           for _ in range(NDT)]
                for k in range(NDT):
                    nc.sync.dma_start(out=w_weffT[k],
                                      in_=weffT[k * 128:(k + 1) * 128, :])
                w_convw = [const.tile([128, 4], F32, tag="wconv", name="wconv",
                                      bufs=NDT) for _ in range(NDT)]
                w_convb = [const.tile([128, 1], F32, tag="wconvb", name="wconvb",
                                      bufs=NDT) for _ in range(NDT)]
                w_dtb = [const.tile([128, 1], F32, tag="wdtb", name="wdtb",
                                    bufs=NDT) for _ in range(NDT)]
                w_dvec = [const.tile([128, 1], F32, tag="wdvec", name="wdvec",
                                     bufs=NDT) for _ in range(NDT)]
                for k in range(NDT):
                    sl = slice(k * 128, (k + 1) * 128)
                    nc.gpsimd.dma_start(out=w_convw[k], in_=convw[sl, :])
                    nc.gpsimd.dma_start(out=w_convb[k], in_=convb[sl, :])
                    nc.scalar.dma_start(out=w_dtb[k], in_=dtb[sl, :])
                    nc.sync.dma_start(out=w_dvec[k], in_=dvec[sl, :])

                # --- in-projection u blocks: u -> u2m seg (as raw u), then
                # conv taps read the seg in place, silu overwrites it with u2.
                # u-proj runs CHUNK-major (c outer) so conv-half0 of every
                # d-tile is ready after half the u-proj work; conv halves are
                # emitted between the chunk rounds. x_dbl c0 needs half0 of
                # all four d-tiles, so this shortens the prologue chain.
                HW2 = T // 2

                def _uproj_round(c):
                    cs = slice(c * CH, (c + 1) * CH)
                    for mb in range(NDT):
                        p, kk = mb // 2, mb % 2
                        seg0 = kk * (T + 1)
                        pmm = ps.tile([128, CH], F32, tag="pmm", name="pmm")
                        for k in range(2):
                            nc.tensor.matmul(pmm, w_inwT[k][:, mb * 128:(mb + 1) * 128],
                                             xn[k][:, cs], start=(k == 0), stop=(k == 1))
                        nc.scalar.copy(
                            out=u2m[p][:, seg0 + c * CH:seg0 + (c + 1) * CH],
                            in_=pmm)

                _uproj_round(0)
                _uproj_round(1)

                def _conv_half(d, half):
                    p, kk = d // 2, d % 2
                    seg0 = kk * (T + 1)
                    useg = u2m[p][:, seg0:seg0 + T]
                    # conv as independent tap products per half (DVE
                    # tensor_scalar, 4x) summed with shifts on the PE into a
                    # 2-bank PSUM half (one matmul per 512 sub-chunk); silu
                    # reads PSUM directly.
                    for half in (half,):
                        g0 = half * HW2
                        pcv = pss.tile([128, HW2], F32, tag="pcv", name="pcv",
                                       bufs=2)
                        for i, k in enumerate((3, 2, 1, 0)):   # shift s = 3-k
                            s = 3 - k
                            a = max(s - g0, 0)
                            mk = big.tile([128, HW2], BF, tag="mtap", name="mk",
                                          bufs=2)
                            if k == 3:
                                nc.vector.tensor_scalar(
                                    out=mk, in0=useg[:, g0:g0 + HW2],
                                    scalar1=w_convw[d][:, 3:4],
                                    scalar2=w_convb[d],
                                    op0=OP.mult, op1=OP.add)
                            else:
                                nc.vector.tensor_scalar(
                                    out=mk[:, a:HW2],
                                    in0=u2m[p][:, seg0 + g0 + a - s:
                                               seg0 + g0 + HW2 - s],
                                    scalar1=w_convw[d][:, k:k + 1],
                                    scalar2=None, op0=OP.mult)
                            for q in range(2):
                                lo = max(a, q * CH)
                                hi = (q + 1) * CH
                                nc.tensor.matmul(pcv[:, lo:hi], ident,
                                                 mk[:, lo:hi],
                                                 start=(i == 0), stop=(i == 3))
                        nc.scalar.activation(u2m[p][:, seg0 + g0:seg0 + g0 + HW2],
                                             pcv, AF.Silu, bias=0.0, scale=1.0)

                for d in range(NDT):
                    _conv_half(d, 0)
                _uproj_round(2)
                _uproj_round(3)
                for d in range(NDT):
                    _conv_half(d, 1)

                # --- x_dbl = xpwT.T @ u2 ([48, T]), pair-0 delta (native
                # Softplus activation, no Exp/Ln scratch dance), and the FULL
                # n=0 pipeline fused per chunk: bc bounce, dA, dBu, chunked
                # scans chained via initial=h[:, prev_last], and yp. The first
                # scan piece starts as soon as chunk 0 of the delta chain
                # lands instead of waiting for the full-width tiles.
                dtBC = const.tile([DR + 2 * DS, T], BF, tag="dtbc", name="dtbc")

                def dual_ap(t, c0, w):
                    """[128, 2, w] AP over both mega segments at col c0."""
                    return bass.AP(tensor=t.tensor, offset=t.offset + c0,
                                   ap=[t.ap[0], [T + 1, 2], [1, w]])

                bc0 = nbp.tile([128, 2 * TP], BF, tag="bcn", name="bcn", bufs=3)
                dA0 = nbp.tile([128, MW], BF, tag="dA", name="dA", bufs=2)
                dBu0 = nbp.tile([128, MW], BF, tag="dBu", name="dBu", bufs=1)
                h0 = nbp.tile([128, MW], BF, tag="h", name="h", bufs=1)
                yp0 = nbp.tile([128, MW], BF, tag="yp", name="yp", bufs=4)
                a_0 = float(avals[0])
                for c in range(NCHUNK):
                    cs = slice(c * CH, (c + 1) * CH)
                    pdb = pss.tile([DR + 2 * DS, CH], F32, tag="px", name="pdb")
                    for d in range(NDT):
                        p, kk = d // 2, d % 2
                        seg = slice(kk * (T + 1) + c * CH, kk * (T + 1) + (c + 1) * CH)
                        nc.tensor.matmul(pdb, w_xpwT[d], u2m[p][:, seg],
                                         start=(d == 0), stop=(d == NDT - 1))
                    nc.scalar.copy(out=dtBC[:, cs], in_=pdb)
                    nc.sync.dma_start(out=bcb[:, cs],
                                      in_=dtBC[DR:DR + 2 * DS, cs])
                    for d in (0, 1):
                        kk = d % 2
                        seg = slice(kk * (T + 1) + c * CH,
                                    kk * (T + 1) + (c + 1) * CH)
                        pda = ps.tile([128, CH], F32, tag="pmm", name="pda")
                        nc.tensor.matmul(pda, w_dtwT[:, d * 128:(d + 1) * 128],
                                         dtBC[0:DR, cs], start=True, stop=True)
                        nc.scalar.activation(wdm[0][:, seg], pda, AF.Exp,
                                             bias=w_dtb[d], scale=1.0)

                # softplus tail: Ln(1+exp) per segment, then the chunked n=0
                # pipeline (wdm fill, bc bounce, dA, dBu, chained scans, yp)
                nc.scalar.activation(dlm[0][:, 0:T], wdm[0][:, 0:T],
                                     AF.Ln, bias=1.0, scale=1.0)
                nc.scalar.activation(dlm[0][:, T + 1:MW - 1],
                                     wdm[0][:, T + 1:MW - 1],
                                     AF.Ln, bias=1.0, scale=1.0)
                a_0 = float(avals[0])
                for c in range(NCHUNK):
                    nc.vector.tensor_tensor(out=dual_ap(wdm[0], c * CH, CH),
                                            in0=dual_ap(dlm[0], c * CH, CH),
                                            in1=dual_ap(u2m[0], c * CH, CH),
                                            op=OP.mult)
                    bcout = bass.AP(tensor=bc0.tensor, offset=bc0.offset + c * CH,
                                    ap=[bc0.ap[0], [TP, 2], [1, CH]])
                    bcin = bcb[0:1, c * CH:(c + 1) * CH]
                    nc.gpsimd.dma_start(
                        out=bcout,
                        in_=bass.AP(tensor=bcin.tensor, offset=bcin.offset,
                                    ap=[[0, 128], [DS * TP, 2], [1, CH]]))
                    nc.scalar.activation(dual_ap(dA0, c * CH, CH),
                                         dual_ap(dlm[0], c * CH, CH),
                                         AF.Exp, bias=0.0, scale=a_0)
                    nc.vector.tensor_tensor(
                        out=dual_ap(dBu0, c * CH, CH),
                        in0=dual_ap(wdm[0], c * CH, CH),
                        in1=bass.AP(tensor=bc0.tensor, offset=bc0.offset + c * CH,
                                    ap=[bc0.ap[0], [0, 2], [1, CH]]),
                        op=OP.mult)
                    for kk in range(2):
                        seg = slice(kk * (T + 1) + c * CH,
                                    kk * (T + 1) + (c + 1) * CH)
                        init = 0.0 if c == 0 else h0[:, kk * (T + 1) + c * CH - 1:
                                                     kk * (T + 1) + c * CH]
                        nc.vector.tensor_tensor_scan(h0[:, seg], dA0[:, seg],
                                                     dBu0[:, seg], init,
                                                     op0=OP.mult, op1=OP.add)
                    nc.vector.tensor_tensor(
                        out=dual_ap(yp0, c * CH, CH),
                        in0=dual_ap(h0, c * CH, CH),
                        in1=bass.AP(tensor=bc0.tensor,
                                    offset=bc0.offset + TP + c * CH,
                                    ap=[bc0.ap[0], [0, 2], [1, CH]]),
                        op=OP.mult)

                # --- delta for the remaining d-tiles: Exp scratch + Ln(1+x)
                def _delta_mm(d):
                    p, kk = d // 2, d % 2
                    for c in range(NCHUNK):
                        seg = slice(kk * (T + 1) + c * CH, kk * (T + 1) + (c + 1) * CH)
                        pda = ps.tile([128, CH], F32, tag="pmm", name="pda")
                        nc.tensor.matmul(pda, w_dtwT[:, d * 128:(d + 1) * 128],
                                         dtBC[0:DR, c * CH:(c + 1) * CH],
                                         start=True, stop=True)
                        nc.scalar.activation(wdm[p][:, seg], pda, AF.Exp,
                                             bias=w_dtb[d], scale=1.0)

                def _delta_fin(p):
                    nc.scalar.activation(dlm[p][:, 0:T], wdm[p][:, 0:T],
                                         AF.Ln, bias=1.0, scale=1.0)
                    nc.scalar.activation(dlm[p][:, T + 1:MW - 1],
                                         wdm[p][:, T + 1:MW - 1],
                                         AF.Ln, bias=1.0, scale=1.0)
                    nc.vector.tensor_mul(wdm[p], dlm[p], u2m[p])

                # --- z-projection blocks, right after pair-0's delta chain.
                # Drains go through DVE tensor_scalar (NOT Act) so the Act
                # queue stays clear for the held dA exps, and the prologue
                # PSUM tiles release early (psy needs all 8 banks). All four
                # silus are applied in place during the scan passes (Act has
                # slack there).
                def _zblock(mb):
                    for c in range(NCHUNK):
                        cs = slice(c * CH, (c + 1) * CH)
                        pmm = ps.tile([128, CH], F32, tag="pmm", name="pmm")
                        for k in range(2):
                            nc.tensor.matmul(pmm, w_inwT[k][:, mb * 128:(mb + 1) * 128],
                                             xn[k][:, cs], start=(k == 0), stop=(k == 1))
                        nc.scalar.copy(out=sz[mb - NDT][:, cs], in_=pmm)

                def emit_n_compute(p, n):
                    a_n = float(avals[n])
                    # one combined broadcast per n: B row then C row
                    bc = nbp.tile([128, 2 * TP], BF, tag="bcn", name="bcn",
                                  bufs=3)
                    nc.gpsimd.dma_start(out=bc, in_=_bc2_ap(bcb, n, TP))
                    dA = nbp.tile([128, MW], BF, tag="dA", name="dA", bufs=2)
                    nc.scalar.activation(dA, dlm[p], AF.Exp, bias=0.0, scale=a_n)
                    dBu = nbp.tile([128, MW], BF, tag="dBu", name="dBu", bufs=1)
                    nc.vector.tensor_tensor(out=dBu, in0=wdm[p],
                                            in1=_rep2_ap(bc, TP), op=OP.mult)
                    h = nbp.tile([128, MW], BF, tag="h", name="h", bufs=1)
                    nc.vector.tensor_tensor_scan(h, dA, dBu, 0.0,
                                                 op0=OP.mult, op1=OP.add)
                    # NOTE: do NOT offload these mults to the Pool engine —
                    # Pool shares SBUF ports with DVE and concurrent Pool
                    # ops slow DVE scans ~1.5x (measured 8.7us -> 12.3us).
                    yp = nbp.tile([128, MW], BF, tag="yp", name="yp", bufs=4)
                    nc.vector.tensor_tensor(out=yp, in0=h,
                                            in1=_rep2_ap(bc, TP, TP), op=OP.mult)
                    return yp

                def emit_n_mm(pyac, n, yp):
                    # the diag(D) skip-mm OPENS each group (emit_d_open), so
                    # n=15 closes it and the gate fires without a close-mm
                    for k in range(2):
                        for c in range(NCHUNK):
                            seg = slice(k * (T + 1) + c * CH,
                                        k * (T + 1) + (c + 1) * CH)
                            nc.tensor.matmul(pyac[k * NCHUNK + c], ident,
                                             yp[:, seg], start=False,
                                             stop=(n == DS - 1))

                def emit_d_open(p, pyac):
                    # u2*D skip term as the accumulation-group STARTER: it
                    # only needs u2m, which is ready long before drain time
                    for c in range(NCHUNK):
                        for k in range(2):
                            d = 2 * p + k
                            seg = slice(k * (T + 1) + c * CH,
                                        k * (T + 1) + (c + 1) * CH)
                            nc.tensor.matmul(pyac[k * NCHUNK + c], w_ddg[d],
                                             u2m[p][:, seg], start=True,
                                             stop=False)

                # held pass-0 computes: n=0 came from the fused chunk loop;
                # n=1..3 are mega ops. PE accumulations deferred to psy.
                held = [yp0, emit_n_compute(0, 1)]
                for mb in range(NDT, 2 * NDT):
                    _zblock(mb)
                _delta_mm(2)
                _delta_mm(3)
                _delta_fin(1)
                held.append(emit_n_compute(0, 2))
                held.append(emit_n_compute(0, 3))

            # --- selective scan: 2 passes over d-pairs ----------------------
            with tc.tile_pool(name="psy", bufs=8, space="PSUM") as psy:
                def mk_drain(p, pyac):
                    # drain (the diag(D) skip opened the group; n=15's accum
                    # mm closed it). Pass 0: Act copies PSUM -> ygc at the
                    # transition (releases the banks for pass 1 with no DVE
                    # work there); the in-place SBUF gate TTs are deferred to
                    # the tail, where DVE idles under the epilogue matmuls.
                    # Pass 1 gates straight from PSUM (shortest tail chain).
                    def _drain():
                        for c in range(NCHUNK):
                            cs = slice(c * CH, (c + 1) * CH)
                            for k in range(2):
                                d = 2 * p + k
                                if p == 0:
                                    nc.scalar.copy(out=ygc[d][:, cs],
                                                   in_=pyac[k * NCHUNK + c])
                                else:
                                    nc.vector.tensor_tensor(
                                        out=ygc[d][:, cs],
                                        in0=pyac[k * NCHUNK + c],
                                        in1=sz[d][:, cs], op=OP.mult)
                    return _drain

                def emit_p0_gates():
                    for c in range(NCHUNK):
                        cs = slice(c * CH, (c + 1) * CH)
                        for d in (0, 1):
                            nc.vector.tensor_tensor(
                                out=ygc[d][:, cs], in0=ygc[d][:, cs],
                                in1=sz[d][:, cs], op=OP.mult)

                pend_drain = None
                for p in range(2):
                    # 8 psum accumulators: (k in pair, chunk) -> [128, 512]
                    pyac = [psy.tile([128, CH], F32, tag="pyac", name="pyac")
                            for _ in range(8)]
                    emit_d_open(p, pyac)
                    n0 = 0
                    if p == 0:
                        for n in range(len(held)):
                            emit_n_mm(pyac, n, held[n])
                        n0 = len(held)
                    for n in range(n0, DS):
                        yp = emit_n_compute(p, n)
                        if pend_drain is not None:
                            # pass-0 drain emitted after pass-1's first
                            # compute: its gate TTs queue behind that scan so
                            # DVE never stalls at the pass boundary
                            pend_drain()
                            pend_drain = None
                        emit_n_mm(pyac, n, yp)
                        # deferred in-place z silus, two per pass, spread so
                        # each hides behind a scan's worth of Act slack
                        if p == 0 and n in (6, 8):
                            dz = (n - 6) // 2
                            nc.scalar.activation(sz[dz], sz[dz], AF.Silu,
                                                 bias=0.0, scale=1.0)
                        if p == 1 and n in (2, 3):
                            dz = n  # sz[2] at n==2, sz[3] at n==3
                            nc.scalar.activation(sz[dz], sz[dz], AF.Silu,
                                                 bias=0.0, scale=1.0)
                    pend_drain = mk_drain(p, pyac)
                pend_drain()
                emit_p0_gates()

            # --- epilogue: fused (fus_w @ out_w) projection, chunked.
            # Act drains PSUM to bf16; output DMAs split across two queues
            # (sync + gpsimd) so the tail DMA drain is not serialized.
            with tc.tile_pool(name="pse", bufs=3, space="PSUM") as pse:
                for c in range(NCHUNK):
                    cs = slice(c * CH, (c + 1) * CH)
                    for mb in range(2):
                        pmf = pse.tile([128, CH], F32, tag="pme", name="pmf")
                        for k in range(NDT):
                            nc.tensor.matmul(pmf, w_weffT[k][:, mb * 128:(mb + 1) * 128],
                                             ygc[k][:, cs], start=(k == 0),
                                             stop=(k == NDT - 1))
                        osb = work.tile([128, CH], BF, tag="osb", name="osb", bufs=2)
                        nc.scalar.copy(out=osb, in_=pmf)
                        h2 = CH // 2
                        nc.sync.dma_start(out=o2[mb * 128:(mb + 1) * 128,
                                                 c * CH:c * CH + h2],
                                          in_=osb[:, 0:h2])
                        nc.gpsimd.dma_start(out=o2[mb * 128:(mb + 1) * 128,
                                                   c * CH + h2:(c + 1) * CH],
                                            in_=osb[:, h2:CH])

    nc.finalize()
    return nc


def _prep_core(xn_b, inp, pfx, direction, fus_w, idh):
    """Host-side input map for one core. xn_b is the pre-normalized x."""
    bf16 = ml_dtypes.bfloat16
    xt = np.ascontiguousarray(xn_b.T)
    if direction:
        xt = np.ascontiguousarray(xt[:, ::-1])
    g = lambda k: np.asarray(inp[pfx + k])
    w_eff = fus_w[:, direction * DM:(direction + 1) * DM].astype(np.float32) @ \
        g("out_w").astype(np.float32)          # [DM, DI]
    m = {
        "xt": xt.astype(bf16),
        "inwT": np.ascontiguousarray(g("in_w").T).astype(bf16),
        "xpwT": np.ascontiguousarray(g("xproj_w").T).astype(bf16),
        "dtwT": np.ascontiguousarray(g("dt_w").T).astype(bf16),
        "weffT": np.ascontiguousarray(w_eff.T).astype(bf16),
        "convw": np.ascontiguousarray(g("conv_w")).astype(np.float32),
        "convb": g("conv_b").reshape(DI, 1).astype(np.float32),
        "dtb": g("dt_b").reshape(DI, 1).astype(np.float32),
        "dvec": g("D").reshape(DI, 1).astype(np.float32),
        "idh": idh,
        "ddg": np.concatenate([np.diag(g("D")[kq * 128:(kq + 1) * 128])
                               for kq in range(NDT)], axis=0).astype(bf16),
    }
    return m


def _run(inputs, trace=False):
    x = np.asarray(inputs["x"], np.float32)
    B = x.shape[0]
    assert x.shape == (4, T, DM), x.shape
    fus_w = np.asarray(inputs["fus_w"], np.float32)
    fus_b = np.asarray(inputs["fus_b"], np.float32)
    norm_w = np.asarray(inputs["norm_w"], np.float32)
    norm_b = np.asarray(inputs["norm_b"], np.float32)
    # layernorm on host (pure input preprocessing, like the transposes)
    mu = x.mean(-1, keepdims=True)
    var = x.var(-1, keepdims=True)
    xnorm = (x - mu) / np.sqrt(var + 1e-5) * norm_w + norm_b
    idh = np.eye(128, dtype=ml_dtypes.bfloat16)

    avals_f = -np.exp(np.asarray(inputs["f_A_log"], np.float32)[0])
    avals_b = -np.exp(np.asarray(inputs["b_A_log"], np.float32)[0])
    assert np.allclose(avals_f, avals_b), "A must match across directions"
    key = avals_f.tobytes()
    if key not in _CACHE:
        _CACHE[key] = _build(avals_f)
    nc = _CACHE[key]

    in_maps = []
    for b in range(B):
        for direction in (0, 1):
            pfx = "b_" if direction else "f_"
            in_maps.append(_prep_core(xnorm[b], inputs, pfx, direction,
                                      fus_w, idh))

    res = run_bass_kernel_spmd(nc, in_maps, list(range(8)), trace=trace)
    out = np.empty((B, T, DM), np.float32)
    for b in range(B):
        of = np.asarray(res.results[2 * b]["o2"], np.float32)
        ob = np.asarray(res.results[2 * b + 1]["o2"], np.float32)[:, ::-1]
        out[b] = (of + ob).T + x[b] + fus_b[None, :]
    return out, res


def kernel(**inputs):
    out, _ = _run(inputs, trace=False)
    return out



# revision 51
# speedup vs baseline: 1.0188x; 1.0034x over previous
"""Bidirectional Mamba block on 8 Trainium2 NeuronCores.

Sharding: 8 cores = 4 batches x 2 directions (fwd/bwd). Each core runs the
full per-(batch, direction) Mamba pipeline on a time-transposed slice
x[b].T (time-flipped for the backward direction), producing its partial
contribution to the fused output projection. Host sums fwd+bwd partials,
adds the residual and fusion bias.

v2 layout: [d (partitions), t (free)], selective scan restructured:
  - n-loop runs in 2 passes over d-PAIRS using mega tiles [128, 4098]
    (two 2048-column d-tile segments + poisoned boundary columns where
    delta=1e9 -> dA=exp(A*1e9)=0 and u2=0 -> dBu=0, so one
    tensor_tensor_scan instruction covers both segments with a clean
    state reset).
  - B/C broadcast tiles are [128, 2049]; the mega elementwise mults read
    them twice via a stride-0 middle AP dim (keeps DVE 2x mode).
  - y = sum_n h_n*C_n accumulated on the TENSOR engine: per n, 8
    identity matmuls [128x128x512] accumulate yp slices into 8 PSUM
    banks (2 d-tiles x 4 t-chunks). DVE no longer does the adds.
  - gate fused with the PSUM drain (diag(D) close matmul + one TT).

v3 scheduling (the scan window is at the DVE floor of ~13.3us/iter =
dBu TT 2.3 + scan 8.7 + yp TT 2.3; HW rates: TT 2x=0.56ns/el,
tensor_scalar 4x=0.30ns/el, scan 2.12ns/col, Act 0.9ns/el):
  - fus_w @ out_w folded on the host -> single-GEMM epilogue.
  - weight DMAs split across the sync/Act/gpsimd descriptor queues so
    u-proj does not queue behind the x loads.
  - n=0 pipeline fused per chunk into the x_dbl loop with CHAINED
    chunk scans (initial=h[:, prev_last]) - the first scan piece
    issues ~15us before the full-width delta would be ready.
  - z-projection emitted between held n1 and the pair-1 delta; PSUM
    drains via Act so the prologue PSUM frees before psy needs banks.
  - pass-0 drain emitted after pass-1's first compute.
  - PSUM->SBUF drains on Act (scalar.copy); output DMAs on 2 queues.
CAUTION: engine times vary ~20% across device allocations (clock
bins); compare runs via the mega-scan duration (8.69us fast bin).
"""

import numpy as np
import ml_dtypes

import concourse.bass as bass
import concourse.bacc as bacc
import concourse.tile as tile
from concourse import mybir
from concourse.bass_utils import run_bass_kernel_spmd

T = 2048
TP = T + 1          # broadcast tile width (padded)
MW = 2 * T + 2      # mega width: [0:T) seg A, T poison, [T+1:2T+1) seg B, 2T+1 poison
DM = 256      # d_model
DI = 512      # d_inner
DS = 16       # d_state
DR = 16       # dt_rank
NCHUNK = 4    # matmul moving-dim chunks of 512
CH = T // NCHUNK
NDT = DI // 128  # 4 d-tiles of 128 partitions

BF = mybir.dt.bfloat16
F32 = mybir.dt.float32
AF = mybir.ActivationFunctionType
OP = mybir.AluOpType

_CACHE = {}


def _bcast_ap(dram_handle, row, col0, width):
    """AP reading dram[row, col0:col0+width] broadcast across 128 partitions."""
    base = dram_handle[row:row + 1, col0:col0 + width]
    return bass.AP(tensor=base.tensor, offset=base.offset,
                   ap=[[0, 128], [1, width]])


def _rep2_ap(tile_, width, col0=0):
    """Free-replicated read of tile_[:, col0:col0+width] twice."""
    return bass.AP(tensor=tile_.tensor, offset=tile_.offset + col0,
                   ap=[tile_.ap[0], [0, 2], [1, width]])


def _bc2_ap(dram_handle, row, width):
    """Broadcast rows `row` and `row+DS` of bcb as one [128, 2*width] read."""
    base = dram_handle[row:row + 1, 0:width]
    return bass.AP(tensor=base.tensor, offset=base.offset,
                   ap=[[0, 128], [DS * width, 2], [1, width]])


def _build(avals):
    nc = bacc.Bacc()

    # --- I/O ---------------------------------------------------------------
    xt = nc.declare_dram_parameter("xt", [DM, T], BF, isOutput=False)
    inwT = nc.declare_dram_parameter("inwT", [DM, 2 * DI], BF, isOutput=False)
    xpwT = nc.declare_dram_parameter("xpwT", [DI, DR + 2 * DS], BF, isOutput=False)
    dtwT = nc.declare_dram_parameter("dtwT", [DR, DI], BF, isOutput=False)
    # weffT = (fus_w_half @ out_w).T — the fusion matmul folded into the
    # output projection on the host, so the epilogue is a single GEMM
    weffT = nc.declare_dram_parameter("weffT", [DI, DM], BF, isOutput=False)
    convw = nc.declare_dram_parameter("convw", [DI, 4], F32, isOutput=False)
    convb = nc.declare_dram_parameter("convb", [DI, 1], F32, isOutput=False)
    dtb = nc.declare_dram_parameter("dtb", [DI, 1], F32, isOutput=False)
    dvec = nc.declare_dram_parameter("dvec", [DI, 1], F32, isOutput=False)
    idh = nc.declare_dram_parameter("idh", [128, 128], BF, isOutput=False)
    ddg = nc.declare_dram_parameter("ddg", [NDT * 128, 128], BF, isOutput=False)
    o2 = nc.declare_dram_parameter("o2", [DM, T], BF, isOutput=True)

    # DRAM scratch for partition-broadcast bounces
    bcb = nc.dram_tensor("bcb", [2 * DS, TP], BF)   # B rows 0..15, C rows 16..31

    with tile.TileContext(nc) as tc:
        with (
            tc.tile_pool(name="const", bufs=1) as const,
            tc.tile_pool(name="big", bufs=2) as big,
            tc.tile_pool(name="pers", bufs=4) as pers,
            tc.tile_pool(name="work", bufs=2) as work,
            tc.tile_pool(name="nb_", bufs=2) as nbp,
        ):
            # --- load x (chunked across DMA queues) -------------------------
            xn = [big.tile([128, T], BF, tag="xn", name="xn", bufs=2)
                  for _ in range(2)]
            for k in range(2):
                for c in range(NCHUNK):
                    cs = slice(c * CH, (c + 1) * CH)
                    nc.sync.dma_start(out=xn[k][:, cs],
                                      in_=xt[k * 128:(k + 1) * 128, cs])


            ident = const.tile([128, 128], BF, tag="ident", name="ident")
            nc.gpsimd.dma_start(out=ident, in_=idh[:, :])
            w_ddg = [const.tile([128, 128], BF, tag="wddg", name="wddg",
                                bufs=NDT) for _ in range(NDT)]
            for kq in range(NDT):
                nc.sync.dma_start(out=w_ddg[kq], in_=ddg[kq * 128:(kq + 1) * 128, :])
            # zero the padded column of the B/C bounce buffer so the
            # broadcast reads a finite value at the mega poison column
            zrow = const.tile([2 * DS, 1], BF, tag="zrow", name="zrow")
            nc.vector.memset(zrow, 0.0)
            nc.sync.dma_start(out=bcb[:, T:TP], in_=zrow)
            # persistent mega tiles (2 d-pairs)
            u2m = [pers.tile([128, MW], BF, tag="u2m", name="u2m", bufs=2)
                   for _ in range(2)]
            dlm = [pers.tile([128, MW], BF, tag="dlm", name="dlm", bufs=2)
                   for _ in range(2)]
            wdm = [pers.tile([128, MW], BF, tag="wdm", name="wdm", bufs=2)
                   for _ in range(2)]
            sz = [pers.tile([128, T], BF, tag="sz", name="sz") for _ in range(NDT)]
            ygc = [pers.tile([128, T], BF, tag="ygc", name="ygc")
                   for _ in range(NDT)]
            # poison columns: delta=1e9, u2=0, wdm=0 at cols T and 2T+1
            # (wdm poisons are memset because the chunked pair-0 fill never
            # touches them, and stale NaN*0 = NaN would break the scan reset)
            for p in range(2):
                nc.vector.memset(dlm[p][:, T:T + 1], 1e9)
                nc.vector.memset(dlm[p][:, MW - 1:MW], 1e9)
                nc.vector.memset(u2m[p][:, T:T + 1], 0.0)
                nc.vector.memset(u2m[p][:, MW - 1:MW], 0.0)
                nc.vector.memset(wdm[p][:, T:T + 1], 0.0)
                nc.vector.memset(wdm[p][:, MW - 1:MW], 0.0)

            def mseg(p, k):
                """Segment slice of mega tile for d-tile index (2*p + k)."""
                return slice(k * (T + 1), k * (T + 1) + T)

            with tc.tile_pool(name="ps", bufs=2, space="PSUM") as ps, \
                 tc.tile_pool(name="pss", bufs=2, space="PSUM") as pss:
                # --- weights/constants ------------------------------------------
                w_inwT = [const.tile([128, 2 * DI], BF, tag="winw", name="winw",
                                     bufs=2) for _ in range(2)]
                for k in range(2):
                    nc.gpsimd.dma_start(out=w_inwT[k], in_=inwT[k * 128:(k + 1) * 128, :])
                w_xpwT = [const.tile([128, DR + 2 * DS], BF, tag="wxpw", name="wxpw",
                                     bufs=NDT) for _ in range(NDT)]
                for k in range(NDT):
                    nc.scalar.dma_start(out=w_xpwT[k], in_=xpwT[k * 128:(k + 1) * 128, :])
                w_dtwT = const.tile([DR, DI], BF, tag="wdtw", name="wdtw")
                nc.scalar.dma_start(out=w_dtwT, in_=dtwT[:, :])
                w_weffT = [const.tile([128, DM], BF, tag="wow", name="wow", bufs=NDT)
                           for _ in range(NDT)]
                for k in range(NDT):
                    nc.sync.dma_start(out=w_weffT[k],
                                      in_=weffT[k * 128:(k + 1) * 128, :])
                w_convw = [const.tile([128, 4], F32, tag="wconv", name="wconv",
                                      bufs=NDT) for _ in range(NDT)]
                w_convb = [const.tile([128, 1], F32, tag="wconvb", name="wconvb",
                                      bufs=NDT) for _ in range(NDT)]
                w_dtb = [const.tile([128, 1], F32, tag="wdtb", name="wdtb",
                                    bufs=NDT) for _ in range(NDT)]
                w_dvec = [const.tile([128, 1], F32, tag="wdvec", name="wdvec",
                                     bufs=NDT) for _ in range(NDT)]
                for k in range(NDT):
                    sl = slice(k * 128, (k + 1) * 128)
                    nc.gpsimd.dma_start(out=w_convw[k], in_=convw[sl, :])
                    nc.gpsimd.dma_start(out=w_convb[k], in_=convb[sl, :])
                    nc.scalar.dma_start(out=w_dtb[k], in_=dtb[sl, :])
                    nc.sync.dma_start(out=w_dvec[k], in_=dvec[sl, :])

                # --- in-projection u blocks: u -> u2m seg (as raw u), then
                # conv taps read the seg in place, silu overwrites it with u2.
                # u-proj runs CHUNK-major (c outer) so conv-half0 of every
                # d-tile is ready after half the u-proj work; conv halves are
                # emitted between the chunk rounds. x_dbl c0 needs half0 of
                # all four d-tiles, so this shortens the prologue chain.
                HW2 = T // 2

                def _uproj_round(c):
                    cs = slice(c * CH, (c + 1) * CH)
                    for mb in range(NDT):
                        p, kk = mb // 2, mb % 2
                        seg0 = kk * (T + 1)
                        pmm = ps.tile([128, CH], F32, tag="pmm", name="pmm")
                        for k in range(2):
                            nc.tensor.matmul(pmm, w_inwT[k][:, mb * 128:(mb + 1) * 128],
                                             xn[k][:, cs], start=(k == 0), stop=(k == 1))
                        nc.scalar.copy(
                            out=u2m[p][:, seg0 + c * CH:seg0 + (c + 1) * CH],
                            in_=pmm)

                _uproj_round(0)
                _uproj_round(1)

                def _conv_half(d, half):
                    p, kk = d // 2, d % 2
                    seg0 = kk * (T + 1)
                    useg = u2m[p][:, seg0:seg0 + T]
                    # conv as independent tap products per half (DVE
                    # tensor_scalar, 4x). Tap sums split across engines: the
                    # prologue is PE-bound now that the drains live on Act,
                    # so d-tiles 0,1 sum on the PE (identity matmuls, silu
                    # reads PSUM) and d-tiles 2,3 tree-sum on DVE.
                    on_pe = d < 2
                    g0 = half * HW2
                    if on_pe:
                        pcv = pss.tile([128, HW2], F32, tag="pcv", name="pcv",
                                       bufs=2)
                    mks = []
                    for i, k in enumerate((3, 2, 1, 0)):   # shift s = 3-k
                        s = 3 - k
                        a = max(s - g0, 0)
                        mk = big.tile([128, HW2], BF, tag="mtap", name="mk",
                                      bufs=4)
                        if k == 3:
                            nc.vector.tensor_scalar(
                                out=mk, in0=useg[:, g0:g0 + HW2],
                                scalar1=w_convw[d][:, 3:4],
                                scalar2=w_convb[d],
                                op0=OP.mult, op1=OP.add)
                        else:
                            if a > 0 and not on_pe:
                                nc.vector.memset(mk[:, 0:a], 0.0)
                            nc.vector.tensor_scalar(
                                out=mk[:, a:HW2],
                                in0=u2m[p][:, seg0 + g0 + a - s:
                                           seg0 + g0 + HW2 - s],
                                scalar1=w_convw[d][:, k:k + 1],
                                scalar2=None, op0=OP.mult)
                        mks.append(mk)
                        if on_pe:
                            for q in range(2):
                                lo = max(a, q * CH)
                                hi = (q + 1) * CH
                                nc.tensor.matmul(pcv[:, lo:hi], ident,
                                                 mk[:, lo:hi],
                                                 start=(i == 0), stop=(i == 3))
                    if on_pe:
                        nc.scalar.activation(u2m[p][:, seg0 + g0:seg0 + g0 + HW2],
                                             pcv, AF.Silu, bias=0.0, scale=1.0)
                    else:
                        c1 = big.tile([128, HW2], BF, tag="csum", name="c1",
                                      bufs=2)
                        nc.vector.tensor_tensor(out=c1, in0=mks[0],
                                                in1=mks[1], op=OP.add)
                        c2 = big.tile([128, HW2], BF, tag="csum", name="c2",
                                      bufs=2)
                        nc.vector.tensor_tensor(out=c2, in0=c1,
                                                in1=mks[2], op=OP.add)
                        c3 = big.tile([128, HW2], BF, tag="csum", name="c3",
                                      bufs=2)
                        nc.vector.tensor_tensor(out=c3, in0=c2,
                                                in1=mks[3], op=OP.add)
                        nc.scalar.activation(u2m[p][:, seg0 + g0:seg0 + g0 + HW2],
                                             c3, AF.Silu, bias=0.0, scale=1.0)

                for d in range(NDT):
                    _conv_half(d, 0)
                _uproj_round(2)
                _uproj_round(3)
                for d in range(NDT):
                    _conv_half(d, 1)

                # --- x_dbl = xpwT.T @ u2 ([48, T]), pair-0 delta (native
                # Softplus activation, no Exp/Ln scratch dance), and the FULL
                # n=0 pipeline fused per chunk: bc bounce, dA, dBu, chunked
                # scans chained via initial=h[:, prev_last], and yp. The first
                # scan piece starts as soon as chunk 0 of the delta chain
                # lands instead of waiting for the full-width tiles.
                dtBC = const.tile([DR + 2 * DS, T], BF, tag="dtbc", name="dtbc")

                def dual_ap(t, c0, w):
                    """[128, 2, w] AP over both mega segments at col c0."""
                    return bass.AP(tensor=t.tensor, offset=t.offset + c0,
                                   ap=[t.ap[0], [T + 1, 2], [1, w]])

                bc0 = nbp.tile([128, 2 * TP], BF, tag="bcn", name="bcn", bufs=2)
                dA0 = nbp.tile([128, MW], BF, tag="dA", name="dA", bufs=2)
                dBu0 = nbp.tile([128, MW], BF, tag="dBu", name="dBu", bufs=1)
                h0 = nbp.tile([128, MW], BF, tag="h", name="h", bufs=1)
                yp0 = nbp.tile([128, MW], BF, tag="yp", name="yp", bufs=4)
                a_0 = float(avals[0])
                for c in range(NCHUNK):
                    cs = slice(c * CH, (c + 1) * CH)
                    pdb = pss.tile([DR + 2 * DS, CH], F32, tag="px", name="pdb")
                    for d in range(NDT):
                        p, kk = d // 2, d % 2
                        seg = slice(kk * (T + 1) + c * CH, kk * (T + 1) + (c + 1) * CH)
                        nc.tensor.matmul(pdb, w_xpwT[d], u2m[p][:, seg],
                                         start=(d == 0), stop=(d == NDT - 1))
                    nc.scalar.copy(out=dtBC[:, cs], in_=pdb)
                    nc.sync.dma_start(out=bcb[:, cs],
                                      in_=dtBC[DR:DR + 2 * DS, cs])
                    for d in (0, 1):
                        kk = d % 2
                        seg = slice(kk * (T + 1) + c * CH,
                                    kk * (T + 1) + (c + 1) * CH)
                        pda = ps.tile([128, CH], F32, tag="pmm", name="pda")
                        nc.tensor.matmul(pda, w_dtwT[:, d * 128:(d + 1) * 128],
                                         dtBC[0:DR, cs], start=True, stop=True)
                        nc.scalar.activation(wdm[0][:, seg], pda, AF.Exp,
                                             bias=w_dtb[d], scale=1.0)

                # softplus tail: Ln(1+exp) per segment, then the chunked n=0
                # pipeline (wdm fill, bc bounce, dA, dBu, chained scans, yp)
                nc.scalar.activation(dlm[0][:, 0:T], wdm[0][:, 0:T],
                                     AF.Ln, bias=1.0, scale=1.0)
                nc.scalar.activation(dlm[0][:, T + 1:MW - 1],
                                     wdm[0][:, T + 1:MW - 1],
                                     AF.Ln, bias=1.0, scale=1.0)
                a_0 = float(avals[0])
                for c in range(NCHUNK):
                    nc.vector.tensor_tensor(out=dual_ap(wdm[0], c * CH, CH),
                                            in0=dual_ap(dlm[0], c * CH, CH),
                                            in1=dual_ap(u2m[0], c * CH, CH),
                                            op=OP.mult)
                    bcout = bass.AP(tensor=bc0.tensor, offset=bc0.offset + c * CH,
                                    ap=[bc0.ap[0], [TP, 2], [1, CH]])
                    bcin = bcb[0:1, c * CH:(c + 1) * CH]
                    nc.gpsimd.dma_start(
                        out=bcout,
                        in_=bass.AP(tensor=bcin.tensor, offset=bcin.offset,
                                    ap=[[0, 128], [DS * TP, 2], [1, CH]]))
                    nc.scalar.activation(dual_ap(dA0, c * CH, CH),
                                         dual_ap(dlm[0], c * CH, CH),
                                         AF.Exp, bias=0.0, scale=a_0)
                    nc.vector.tensor_tensor(
                        out=dual_ap(dBu0, c * CH, CH),
                        in0=dual_ap(wdm[0], c * CH, CH),
                        in1=bass.AP(tensor=bc0.tensor, offset=bc0.offset + c * CH,
                                    ap=[bc0.ap[0], [0, 2], [1, CH]]),
                        op=OP.mult)
                    for kk in range(2):
                        seg = slice(kk * (T + 1) + c * CH,
                                    kk * (T + 1) + (c + 1) * CH)
                        init = 0.0 if c == 0 else h0[:, kk * (T + 1) + c * CH - 1:
                                                     kk * (T + 1) + c * CH]
                        nc.vector.tensor_tensor_scan(h0[:, seg], dA0[:, seg],
                                                     dBu0[:, seg], init,
                                                     op0=OP.mult, op1=OP.add)
                    nc.vector.tensor_tensor(
                        out=dual_ap(yp0, c * CH, CH),
                        in0=dual_ap(h0, c * CH, CH),
                        in1=bass.AP(tensor=bc0.tensor,
                                    offset=bc0.offset + TP + c * CH,
                                    ap=[bc0.ap[0], [0, 2], [1, CH]]),
                        op=OP.mult)

                # --- delta for the remaining d-tiles: Exp scratch + Ln(1+x)
                def _delta_mm(d):
                    p, kk = d // 2, d % 2
                    for c in range(NCHUNK):
                        seg = slice(kk * (T + 1) + c * CH, kk * (T + 1) + (c + 1) * CH)
                        pda = ps.tile([128, CH], F32, tag="pmm", name="pda")
                        nc.tensor.matmul(pda, w_dtwT[:, d * 128:(d + 1) * 128],
                                         dtBC[0:DR, c * CH:(c + 1) * CH],
                                         start=True, stop=True)
                        nc.scalar.activation(wdm[p][:, seg], pda, AF.Exp,
                                             bias=w_dtb[d], scale=1.0)

                def _delta_fin(p):
                    nc.scalar.activation(dlm[p][:, 0:T], wdm[p][:, 0:T],
                                         AF.Ln, bias=1.0, scale=1.0)
                    nc.scalar.activation(dlm[p][:, T + 1:MW - 1],
                                         wdm[p][:, T + 1:MW - 1],
                                         AF.Ln, bias=1.0, scale=1.0)
                    nc.vector.tensor_mul(wdm[p], dlm[p], u2m[p])

                # --- z-projection blocks, right after pair-0's delta chain.
                # Drains go through DVE tensor_scalar (NOT Act) so the Act
                # queue stays clear for the held dA exps, and the prologue
                # PSUM tiles release early (psy needs all 8 banks). All four
                # silus are applied in place during the scan passes (Act has
                # slack there).
                def _zblock(mb):
                    for c in range(NCHUNK):
                        cs = slice(c * CH, (c + 1) * CH)
                        pmm = ps.tile([128, CH], F32, tag="pmm", name="pmm")
                        for k in range(2):
                            nc.tensor.matmul(pmm, w_inwT[k][:, mb * 128:(mb + 1) * 128],
                                             xn[k][:, cs], start=(k == 0), stop=(k == 1))
                        nc.scalar.copy(out=sz[mb - NDT][:, cs], in_=pmm)

                def emit_n_compute(p, n):
                    a_n = float(avals[n])
                    # one combined broadcast per n: B row then C row
                    bc = nbp.tile([128, 2 * TP], BF, tag="bcn", name="bcn",
                                  bufs=2)
                    nc.gpsimd.dma_start(out=bc, in_=_bc2_ap(bcb, n, TP))
                    dA = nbp.tile([128, MW], BF, tag="dA", name="dA", bufs=2)
                    nc.scalar.activation(dA, dlm[p], AF.Exp, bias=0.0, scale=a_n)
                    dBu = nbp.tile([128, MW], BF, tag="dBu", name="dBu", bufs=1)
                    nc.vector.tensor_tensor(out=dBu, in0=wdm[p],
                                            in1=_rep2_ap(bc, TP), op=OP.mult)
                    h = nbp.tile([128, MW], BF, tag="h", name="h", bufs=1)
                    nc.vector.tensor_tensor_scan(h, dA, dBu, 0.0,
                                                 op0=OP.mult, op1=OP.add)
                    # NOTE: do NOT offload these mults to the Pool engine —
                    # Pool shares SBUF ports with DVE and concurrent Pool
                    # ops slow DVE scans ~1.5x (measured 8.7us -> 12.3us).
                    yp = nbp.tile([128, MW], BF, tag="yp", name="yp", bufs=4)
                    nc.vector.tensor_tensor(out=yp, in0=h,
                                            in1=_rep2_ap(bc, TP, TP), op=OP.mult)
                    return yp

                def emit_n_mm(pyac, n, yp):
                    # the diag(D) skip-mm OPENS each group (emit_d_open), so
                    # n=15 closes it and the gate fires without a close-mm
                    for k in range(2):
                        for c in range(NCHUNK):
                            seg = slice(k * (T + 1) + c * CH,
                                        k * (T + 1) + (c + 1) * CH)
                            nc.tensor.matmul(pyac[k * NCHUNK + c], ident,
                                             yp[:, seg], start=False,
                                             stop=(n == DS - 1))

                def emit_d_open(p, pyac):
                    # u2*D skip term as the accumulation-group STARTER: it
                    # only needs u2m, which is ready long before drain time
                    for c in range(NCHUNK):
                        for k in range(2):
                            d = 2 * p + k
                            seg = slice(k * (T + 1) + c * CH,
                                        k * (T + 1) + (c + 1) * CH)
                            nc.tensor.matmul(pyac[k * NCHUNK + c], w_ddg[d],
                                             u2m[p][:, seg], start=True,
                                             stop=False)

                # held pass-0 computes: n=0 came from the fused chunk loop;
                # n=1..3 are mega ops. PE accumulations deferred to psy.
                held = [yp0, emit_n_compute(0, 1)]
                for mb in range(NDT, 2 * NDT):
                    _zblock(mb)
                _delta_mm(2)
                _delta_mm(3)
                _delta_fin(1)
                held.append(emit_n_compute(0, 2))
                held.append(emit_n_compute(0, 3))

            # --- selective scan: 2 passes over d-pairs ----------------------
            with tc.tile_pool(name="psy", bufs=8, space="PSUM") as psy:
                def mk_drain(p, pyac):
                    # drain (the diag(D) skip opened the group; n=15's accum
                    # mm closed it). Pass 0: Act copies PSUM -> ygc at the
                    # transition (releases the banks for pass 1 with no DVE
                    # work there); the in-place SBUF gate TTs are deferred to
                    # the tail, where DVE idles under the epilogue matmuls.
                    # Pass 1 gates straight from PSUM (shortest tail chain).
                    def _drain():
                        for c in range(NCHUNK):
                            cs = slice(c * CH, (c + 1) * CH)
                            for k in range(2):
                                d = 2 * p + k
                                if p == 0:
                                    nc.scalar.copy(out=ygc[d][:, cs],
                                                   in_=pyac[k * NCHUNK + c])
                                else:
                                    nc.vector.tensor_tensor(
                                        out=ygc[d][:, cs],
                                        in0=pyac[k * NCHUNK + c],
                                        in1=sz[d][:, cs], op=OP.mult)
                    return _drain

                def emit_p0_gates():
                    for c in range(NCHUNK):
                        cs = slice(c * CH, (c + 1) * CH)
                        for d in (0, 1):
                            nc.vector.tensor_tensor(
                                out=ygc[d][:, cs], in0=ygc[d][:, cs],
                                in1=sz[d][:, cs], op=OP.mult)

                pend_drain = None
                for p in range(2):
                    # 8 psum accumulators: (k in pair, chunk) -> [128, 512]
                    pyac = [psy.tile([128, CH], F32, tag="pyac", name="pyac")
                            for _ in range(8)]
                    emit_d_open(p, pyac)
                    n0 = 0
                    if p == 0:
                        for n in range(len(held)):
                            emit_n_mm(pyac, n, held[n])
                        n0 = len(held)
                    for n in range(n0, DS):
                        yp = emit_n_compute(p, n)
                        if pend_drain is not None:
                            # pass-0 drain emitted after pass-1's first
                            # compute: its gate TTs queue behind that scan so
                            # DVE never stalls at the pass boundary
                            pend_drain()
                            pend_drain = None
                        emit_n_mm(pyac, n, yp)
                        # deferred in-place z silus, two per pass, spread so
                        # each hides behind a scan's worth of Act slack
                        if p == 0 and n in (6, 8):
                            dz = (n - 6) // 2
                            nc.scalar.activation(sz[dz], sz[dz], AF.Silu,
                                                 bias=0.0, scale=1.0)
                        if p == 1 and n in (2, 3):
                            dz = n  # sz[2] at n==2, sz[3] at n==3
                            nc.scalar.activation(sz[dz], sz[dz], AF.Silu,
                                                 bias=0.0, scale=1.0)
                    pend_drain = mk_drain(p, pyac)
                pend_drain()
                emit_p0_gates()

            # --- epilogue: fused (fus_w @ out_w) projection, chunked.
            # Act drains PSUM to bf16; output DMAs split across two queues
            # (sync + gpsimd) so the tail DMA drain is not serialized.
            with tc.tile_pool(name="pse", bufs=3, space="PSUM") as pse:
                for c in range(NCHUNK):
                    cs = slice(c * CH, (c + 1) * CH)
                    for mb in range(2):
                        pmf = pse.tile([128, CH], F32, tag="pme", name="pmf")
                        for k in range(NDT):
                            nc.tensor.matmul(pmf, w_weffT[k][:, mb * 128:(mb + 1) * 128],
                                             ygc[k][:, cs], start=(k == 0),
                                             stop=(k == NDT - 1))
                        osb = work.tile([128, CH], BF, tag="osb", name="osb", bufs=2)
                        nc.scalar.copy(out=osb, in_=pmf)
                        h2 = CH // 2
                        nc.sync.dma_start(out=o2[mb * 128:(mb + 1) * 128,
                                                 c * CH:c * CH + h2],
                                          in_=osb[:, 0:h2])
                        nc.gpsimd.dma_start(out=o2[mb * 128:(mb + 1) * 128,
                                                   c * CH + h2:(c + 1) * CH],
                                            in_=osb[:, h2:CH])

    nc.finalize()
    return nc


def _prep_core(xn_b, inp, pfx, direction, fus_w, idh):
    """Host-side input map for one core. xn_b is the pre-normalized x."""
    bf16 = ml_dtypes.bfloat16
    xt = np.ascontiguousarray(xn_b.T)
    if direction:
        xt = np.ascontiguousarray(xt[:, ::-1])
    g = lambda k: np.asarray(inp[pfx + k])
    w_eff = fus_w[:, direction * DM:(direction + 1) * DM].astype(np.float32) @ \
        g("out_w").astype(np.float32)          # [DM, DI]
    m = {
        "xt": xt.astype(bf16),
        "inwT": np.ascontiguousarray(g("in_w").T).astype(bf16),
        "xpwT": np.ascontiguousarray(g("xproj_w").T).astype(bf16),
        "dtwT": np.ascontiguousarray(g("dt_w").T).astype(bf16),
        "weffT": np.ascontiguousarray(w_eff.T).astype(bf16),
        "convw": np.ascontiguousarray(g("conv_w")).astype(np.float32),
        "convb": g("conv_b").reshape(DI, 1).astype(np.float32),
        "dtb": g("dt_b").reshape(DI, 1).astype(np.float32),
        "dvec": g("D").reshape(DI, 1).astype(np.float32),
        "idh": idh,
        "ddg": np.concatenate([np.diag(g("D")[kq * 128:(kq + 1) * 128])
                               for kq in range(NDT)], axis=0).astype(bf16),
    }
    return m


def _run(inputs, trace=False):
    x = np.asarray(inputs["x"], np.float32)
    B = x.shape[0]
    assert x.shape == (4, T, DM), x.shape
    fus_w = np.asarray(inputs["fus_w"], np.float32)
    fus_b = np.asarray(inputs["fus_b"], np.float32)
    norm_w = np.asarray(inputs["norm_w"], np.float32)
    norm_b = np.asarray(inputs["norm_b"], np.float32)
    # layernorm on host (pure input preprocessing, like the transposes)
    mu = x.mean(-1, keepdims=True)
    var = x.var(-1, keepdims=True)
    xnorm = (x - mu) / np.sqrt(var + 1e-5) * norm_w + norm_b
    idh = np.eye(128, dtype=ml_dtypes.bfloat16)

    avals_f = -np.exp(np.asarray(inputs["f_A_log"], np.float32)[0])
    avals_b = -np.exp(np.asarray(inputs["b_A_log"], np.float32)[0])
    assert np.allclose(avals_f, avals_b), "A must match across directions"
    key = avals_f.tobytes()
    if key not in _CACHE:
        _CACHE[key] = _build(avals_f)
    nc = _CACHE[key]

    in_maps = []
    for b in range(B):
        for direction in (0, 1):
            pfx = "b_" if direction else "f_"
            in_maps.append(_prep_core(xnorm[b], inputs, pfx, direction,
                                      fus_w, idh))

    res = run_bass_kernel_spmd(nc, in_maps, list(range(8)), trace=trace)
    out = np.empty((B, T, DM), np.float32)
    for b in range(B):
        of = np.asarray(res.results[2 * b]["o2"], np.float32)
        ob = np.asarray(res.results[2 * b + 1]["o2"], np.float32)[:, ::-1]
        out[b] = (of + ob).T + x[b] + fus_b[None, :]
    return out, res


def kernel(**inputs):
    out, _ = _run(inputs, trace=False)
    return out

